# revision 41
# baseline (speedup 1.0000x reference)
"""Trainium2 Bass kernel for DiffusionPropers (gnn_message_passing), v5.

v3 data path (512B table slabs: per-atom y_k = enc @ W0_k in bf16 plus
f32-bitcast coords, SWDGE dma_gather / dma_scatter_add) with:
  - num_swdge_queues=4: the 4 per-chunk gathers ride 4 parallel DMA
    rings (v3 serialized all 6 ring transfers on one ring at ~7.6us
    each); scatters rotate across rings.
  - phase 0 table writes split across 4 HWDGE/SWDGE engine queues, and
    coords copies batched per span instead of per block.
  - v4's transposed MLP: Z summed on DVE then transposed per block once;
    h1 = identity-broadcast(Z) + masked-wg @ geoT with the per-t bias
    applied by the activation; ti-pair emission so PE/Act overlap.
"""
import numpy as np
import ml_dtypes

# ---------------- compile-time constants (hardcoded problem shape) --------
N_ATOMS = 25000
NA = 25088              # padded atoms (196 * 128)
P_TOT = 100000
T_STEPS = 4
D = 128
N_CORES = 8
PPC = 12500             # real props per core
PPCT = 12544            # padded props per core (98 tiles of 128)
NTILES = PPCT // 128    # 98
CH = 896                # props per gather/scatter call (SWDGE ring limit)
NCHUNK = PPCT // CH     # 14
CBLK = CH // 128        # 7
HCOL = CH // 2          # 448 cols per PSUM-bank half
SLAB = 256              # bf16 elems per table slab (512B)
DUMP = NA               # scatter dump row
A_ROWS = NA + 8         # accumulator rows (incl. dump)
A_COLS = 64             # 256B stride for scatter
LEAKY = 0.001

_BF16 = ml_dtypes.bfloat16

_compiled = None


# ------------------------- host-side helpers ------------------------------

def _wrap_idxs(idx: np.ndarray) -> np.ndarray:
    """[n] int -> [128, n/16] int16, wrapped in 16 partitions, replicated x8."""
    n = idx.shape[0]
    assert n % 16 == 0
    w = idx.reshape(-1, 16).T.astype(np.int16)
    return np.tile(w, (8, 1))


def _order_props(props: np.ndarray, n_real: int, seed: int = 0) -> np.ndarray:
    """Order PPCT props so that within every aligned CH-chunk the p0 targets
    are distinct and the p3 targets are distinct (scatter-add race freedom)."""
    n = props.shape[0]
    rng = np.random.default_rng(seed)
    for attempt in range(50):
        perm = rng.permutation(n_real)
        buckets: list[list[int]] = [[] for _ in range(NCHUNK)]
        used0: list[set] = [set() for _ in range(NCHUNK)]
        used3: list[set] = [set() for _ in range(NCHUNK)]
        fail = []
        start = 0
        for j in perm:
            a0 = int(props[j, 0])
            a3 = int(props[j, 3])
            for d in range(NCHUNK):
                b = (start + d) % NCHUNK
                if (len(buckets[b]) < CH and a0 not in used0[b]
                        and a3 not in used3[b]):
                    buckets[b].append(int(j))
                    used0[b].add(a0)
                    used3[b].add(a3)
                    break
            else:
                fail.append(int(j))
            start = (start + 1) % NCHUNK
        if fail:
            continue
        pads = list(range(n_real, n))
        for b in range(NCHUNK):
            while len(buckets[b]) < CH:
                buckets[b].append(pads.pop())
        assert not pads
        order = [j for b in buckets for j in b]
        return np.array(order, dtype=np.int64)
    raise RuntimeError("prop ordering failed")


# ------------------------- device kernel build ----------------------------

def _build():
    import concourse.bass as bass
    import concourse.bacc as bacc
    import concourse.mybir as mybir
    import concourse.tile as tile
    from concourse.masks import make_identity
    from concourse.library_config import mlp as mlp_lib

    F32 = mybir.dt.float32
    BF16 = mybir.dt.bfloat16
    I16 = mybir.dt.int16
    AF = mybir.ActivationFunctionType

    nc = bacc.Bacc("TRN2", target_bir_lowering=False, debug=False,
                   num_devices=N_CORES, num_swdge_queues=4)

    # ---- I/O ----
    encT = nc.dram_tensor("encT", [D, NA], BF16, kind="ExternalInput")
    coordsb = nc.dram_tensor("coordsb", [NA, 24], BF16, kind="ExternalInput")
    w0all = nc.dram_tensor("w0all", [D, 512], BF16, kind="ExternalInput")
    wg4 = nc.dram_tensor("wg4", [16, 4, 128], BF16, kind="ExternalInput")
    w1 = nc.dram_tensor("w1", [D, D], BF16, kind="ExternalInput")
    w2 = nc.dram_tensor("w2", [D, D], BF16, kind="ExternalInput")
    w3 = nc.dram_tensor("w3", [D, 2], BF16, kind="ExternalInput")
    bx1 = nc.dram_tensor("bx1", [D, 4], F32, kind="ExternalInput")
    bias12 = nc.dram_tensor("bias12", [D, 2], F32, kind="ExternalInput")
    b3h = nc.dram_tensor("b3h", [D, 2], F32, kind="ExternalInput")
    gidx = nc.dram_tensor("gidx", [128, 4 * (PPCT // 16)], I16,
                          kind="ExternalInput")
    sidx = nc.dram_tensor("sidx", [128, 2 * (PPCT // 16)], I16,
                          kind="ExternalInput")
    A0 = nc.dram_tensor("A0", [A_ROWS, A_COLS], F32, kind="ExternalOutput")
    A3 = nc.dram_tensor("A3", [A_ROWS, A_COLS], F32, kind="ExternalOutput")
    Tt = nc.dram_tensor("Tt", [NA, 4, SLAB], BF16)   # internal table

    GI = PPCT // 16     # 784
    GC = CH // 16       # 56

    with tile.TileContext(nc) as tc:
        with (
            tc.tile_pool(name="const", bufs=1) as cpool,
        ):
            nc.gpsimd.load_library(mlp_lib)

            # ---- constants ----
            ibf = cpool.tile([128, 128], BF16)
            make_identity(nc, ibf[:])
            if32 = cpool.tile([128, 128], F32)
            make_identity(nc, if32[:])
            zero_b = cpool.tile([128, 1], F32)
            nc.vector.memset(zero_b[:], 0.0)
            eps_b = cpool.tile([128, 1], F32)
            nc.vector.memset(eps_b[:], 1e-12)
            negh = cpool.tile([128, 1], F32)
            nc.vector.memset(negh[:], -0.5)
            posh = cpool.tile([128, 1], F32)
            nc.vector.memset(posh[:], 0.5)

            w0t = cpool.tile([D, 512], BF16)
            nc.sync.dma_start(out=w0t[:], in_=w0all[:])
            wgt = cpool.tile([16, 4, 128], BF16)
            nc.sync.dma_start(out=wgt[:], in_=wg4[:])
            w1t = cpool.tile([D, D], BF16)
            nc.sync.dma_start(out=w1t[:], in_=w1[:])
            w2t = cpool.tile([D, D], BF16)
            nc.sync.dma_start(out=w2t[:], in_=w2[:])
            w3t = cpool.tile([D, 2], BF16)
            nc.sync.dma_start(out=w3t[:], in_=w3[:])
            bx1t = cpool.tile([D, 4], F32)
            nc.sync.dma_start(out=bx1t[:], in_=bx1[:])
            b12t = cpool.tile([D, 2], F32)
            nc.sync.dma_start(out=b12t[:], in_=bias12[:])
            b3t = cpool.tile([D, 2], F32)
            nc.sync.dma_start(out=b3t[:], in_=b3h[:])
            gixt = cpool.tile([128, 4 * GI], I16)
            nc.sync.dma_start(out=gixt[:], in_=gidx[:])
            sixt = cpool.tile([128, 2 * GI], I16)
            nc.sync.dma_start(out=sixt[:], in_=sidx[:])

            # ================= Phase 0: build table =================
            SC = 2048
            with (
                tc.tile_pool(name="p0", bufs=3) as p0pool,
                tc.tile_pool(name="p0ps", bufs=8, space="PSUM") as p0ps,
            ):
                cob = p0pool.tile([128, NA // 128, 24], BF16, tag="cob")
                nc.sync.dma_start(
                    out=cob[:],
                    in_=coordsb[:].rearrange("(b p) c -> p b c", p=128))
                nsc = NA // SC
                rem = NA - nsc * SC
                spans = [(i * SC, SC) for i in range(nsc)]
                if rem:
                    spans.append((nsc * SC, rem))
                wengines = [nc.sync, nc.scalar]
                for si, (base, ln) in enumerate(spans):
                    et = p0pool.tile([128, SC], BF16, tag="et")
                    leng = nc.scalar if si % 2 == 0 else nc.sync
                    leng.dma_start(out=et[:, :ln], in_=encT[:, base:base + ln])
                    asm = p0pool.tile([128, SC // 128, 4, SLAB], BF16,
                                      tag="asm")
                    for s in range(ln // 128):
                        ps = p0ps.tile([128, 512], F32, tag="yps")
                        nc.tensor.matmul(ps[:], lhsT=et[:, s * 128:(s + 1) * 128],
                                         rhs=w0t[:], start=True, stop=True)
                        psv = ps[:].rearrange("p (a b) -> p a b", a=4)
                        if s % 2 == 0:
                            nc.scalar.activation(asm[:, s, :, 0:128], psv, AF.Copy)
                        else:
                            nc.vector.tensor_copy(asm[:, s, :, 0:128], psv)
                    # coords: one batched copy per (span, k), on gpsimd
                    nblk = ln // 128
                    blk0 = base // 128
                    for k in range(4):
                        nc.gpsimd.tensor_copy(
                            asm[:, 0:nblk, k, 128:152],
                            cob[:, blk0:blk0 + nblk, :])
                    # one contiguous full-slab write per span (bytes
                    # 152:256 of each slab are never read by compute)
                    wengines[si % 2].dma_start(
                        out=Tt[base:base + ln, :, :].rearrange(
                            "(s p) k e -> p s k e", p=128),
                        in_=asm[:, :nblk, :, :])

            # ================= Phase 1: main loop =================
            with (
                tc.tile_pool(name="gat", bufs=3) as gpool,
                tc.tile_pool(name="mlp", bufs=2) as mpool,
                tc.tile_pool(name="geo", bufs=2) as geopool,
                tc.tile_pool(name="cto", bufs=3) as ctpool,
                tc.tile_pool(name="hps", bufs=2, space="PSUM") as hps,
                tc.tile_pool(name="msc", bufs=2, space="PSUM") as mscps,
                tc.tile_pool(name="dt", bufs=1, space="PSUM") as dtps,
            ):
                Gof = {}
                ctof = {}
                qctr = [0]

                def next_q():
                    q = qctr[0] % 4
                    qctr[0] += 1
                    return q

                def do_gather(c):
                    G = []
                    for k in range(4):
                        g = gpool.tile([128, CBLK, SLAB], BF16, tag=f"g{k}")
                        nc.gpsimd.dma_gather(
                            g[:], Tt[:, k, :],
                            gixt[:, k * GI + c * GC:
                                 k * GI + (c + 1) * GC],
                            CH, CH, SLAB, elem_step=4 * SLAB,
                            queue_num=next_q())
                        G.append(g)
                    Gof[c] = G

                front_of = {}

                def do_front(c):
                    G = Gof[c]
                    cco = [G[k][:, :, 128:152].bitcast(F32) for k in range(4)]

                    # ---- Z sum on DVE (bf16) ----
                    Zp = mpool.tile([128, CBLK, 128], BF16, tag="Zp")
                    nc.vector.tensor_add(Zp[:], G[0][:, :, 0:128],
                                         G[1][:, :, 0:128])
                    nc.vector.tensor_add(Zp[:], Zp[:], G[2][:, :, 0:128])
                    nc.vector.tensor_add(Zp[:], Zp[:], G[3][:, :, 0:128])

                    # ---- Z^T via per-block transpose matmuls ----
                    zbf = mpool.tile([128, CH], BF16, tag="zbf")
                    for hb, nb in ((0, 4), (4, 3)):
                        zps = mscps.tile([128, 512], F32, tag="msc")
                        for b in range(nb):
                            nc.tensor.matmul(
                                zps[:, b * 128:(b + 1) * 128],
                                lhsT=Zp[:, hb + b, :], rhs=ibf[:],
                                start=True, stop=True)
                        nc.vector.tensor_copy(
                            zbf[:, hb * 128:(hb + nb) * 128],
                            zps[:, 0:nb * 128])

                    # ---- geometry (props layout) ----
                    u1 = geopool.tile([128, CBLK, 12], F32, tag="u1")
                    u2 = geopool.tile([128, CBLK, 12], F32, tag="u2")
                    u3 = geopool.tile([128, CBLK, 12], F32, tag="u3")
                    dr = geopool.tile([128, CBLK, 12], F32, tag="dr")
                    nc.vector.tensor_sub(u1[:], cco[1], cco[0])
                    nc.vector.tensor_sub(u2[:], cco[2], cco[1])
                    nc.vector.tensor_sub(u3[:], cco[3], cco[2])
                    nc.vector.tensor_sub(dr[:], cco[0], cco[3])

                    def cross(out, a, b):
                        tmp = geopool.tile([128, CBLK, 4], F32, tag="ctmp")
                        for x in range(3):
                            y, z = (x + 1) % 3, (x + 2) % 3
                            nc.vector.tensor_mul(tmp[:], a[:, :, y::3],
                                                 b[:, :, z::3])
                            nc.vector.tensor_mul(out[:, :, x::3],
                                                 a[:, :, z::3], b[:, :, y::3])
                            nc.vector.tensor_sub(out[:, :, x::3], tmp[:],
                                                 out[:, :, x::3])

                    cr12 = geopool.tile([128, CBLK, 12], F32, tag="cr12")
                    cr23 = geopool.tile([128, CBLK, 12], F32, tag="cr23")
                    cross(cr12, u1, u2)
                    cross(cr23, u2, u3)

                    def dot3(out, a, b, tmp):
                        nc.vector.tensor_mul(tmp[:], a[:], b[:])
                        nc.vector.tensor_add(out[:], tmp[:, :, 0::3],
                                             tmp[:, :, 1::3])
                        nc.vector.tensor_add(out[:], out[:], tmp[:, :, 2::3])

                    tmp12 = geopool.tile([128, CBLK, 12], F32, tag="tmp12")
                    n2 = geopool.tile([128, CBLK, 4], F32, tag="n2")
                    dot3(n2, u2, u2, tmp12)
                    nc.scalar.activation(n2[:], n2[:], AF.Sqrt, bias=zero_b[:])
                    sn = geopool.tile([128, CBLK, 4], F32, tag="sn")
                    dot3(sn, u1, cr23, tmp12)
                    nc.vector.tensor_mul(sn[:], sn[:], n2[:])
                    cn = geopool.tile([128, CBLK, 4], F32, tag="cn")
                    dot3(cn, cr12, cr23, tmp12)
                    hy = geopool.tile([128, CBLK, 4], F32, tag="hy")
                    t2 = geopool.tile([128, CBLK, 4], F32, tag="t2")
                    nc.vector.tensor_mul(hy[:], sn[:], sn[:])
                    nc.vector.tensor_mul(t2[:], cn[:], cn[:])
                    nc.vector.tensor_add(hy[:], hy[:], t2[:])
                    nc.scalar.activation(hy[:], hy[:], AF.Sqrt, bias=eps_b[:])
                    rh = geopool.tile([128, CBLK, 4], F32, tag="rh")
                    nc.vector.reciprocal(rh[:], hy[:])
                    dl = geopool.tile([128, CBLK, 4], F32, tag="dl")
                    dot3(dl, dr, dr, tmp12)
                    nc.scalar.activation(dl[:], dl[:], AF.Sqrt, bias=eps_b[:])
                    rdl = geopool.tile([128, CBLK, 4], F32, tag="rdl")
                    nc.vector.reciprocal(rdl[:], dl[:])
                    dh = geopool.tile([128, CBLK, 12], F32, tag="dh")
                    for x in range(3):
                        nc.vector.tensor_mul(dh[:, :, x::3], dr[:, :, x::3],
                                             rdl[:])
                    # geo features per (prop, t): [sin, cos, dl, pad]
                    geo = geopool.tile([128, CBLK, 16], F32, tag="geo")
                    nc.vector.memset(geo[:, :, 3::4], 0.0)
                    nc.vector.tensor_mul(geo[:, :, 0::4], sn[:], rh[:])
                    nc.vector.tensor_mul(geo[:, :, 1::4], cn[:], rh[:])
                    nc.vector.tensor_copy(geo[:, :, 2::4], dl[:])

                    # geoT [16, 896]
                    geoT = mpool.tile([16, CH], BF16, tag="geoT")
                    for hb, nb in ((0, 4), (4, 3)):
                        gtp = mscps.tile([128, 512], F32, tag="msc")
                        for b in range(nb):
                            nc.tensor.matmul(
                                gtp[0:16, b * 128:(b + 1) * 128],
                                lhsT=geo[:, hb + b, :],
                                rhs=if32[:], is_transpose=True,
                                start=True, stop=True)
                        nc.vector.tensor_copy(
                            geoT[:, hb * 128:(hb + nb) * 128],
                            gtp[0:16, 0:nb * 128])

                    front_of[c] = (zbf, geoT, dh)

                def do_mlp(c):
                    zbf, geoT, dh = front_of[c]

                    # ---- per-t MLP (ti pairs, layer-major in pair) ----
                    dtc = dtps.tile([128, CBLK, 4, 2], F32, tag="dtc")

                    def mm_h1(ti):
                        h1 = hps.tile([128, 2, 512], F32, tag="h")
                        for h in range(2):
                            sl = slice(h * HCOL, (h + 1) * HCOL)
                            nc.tensor.matmul(
                                h1[:, h, 0:HCOL], lhsT=ibf[:],
                                rhs=zbf[:, sl], start=True, stop=False)
                            nc.tensor.matmul(
                                h1[:, h, 0:HCOL],
                                lhsT=wgt[:, ti, :],
                                rhs=geoT[:, sl],
                                start=False, stop=True)
                        return h1

                    def mm_layer(w, x):
                        hp = hps.tile([128, 2, 512], F32, tag="h")
                        for h in range(2):
                            nc.tensor.matmul(hp[:, h, 0:HCOL], lhsT=w[:],
                                             rhs=x[:, h, :],
                                             start=True, stop=True)
                        return hp

                    def act(hp, bias, tag):
                        x = mpool.tile([128, 2, HCOL], BF16, tag=tag)
                        nc.scalar.activation(x[:], hp[:, :, 0:HCOL],
                                             AF.Prelu, bias=bias, alpha=LEAKY)
                        return x

                    for t0 in (0, 2):
                        h1a = mm_h1(t0)
                        h1b = mm_h1(t0 + 1)
                        x1a = act(h1a, bx1t[:, t0:t0 + 1], "x1a")
                        x1b = act(h1b, bx1t[:, t0 + 1:t0 + 2], "x1b")
                        h2a = mm_layer(w1t, x1a)
                        h2b = mm_layer(w1t, x1b)
                        x2a = act(h2a, b12t[:, 0:1], "x2a")
                        x2b = act(h2b, b12t[:, 0:1], "x2b")
                        h3a = mm_layer(w2t, x2a)
                        h3b = mm_layer(w2t, x2b)
                        for ti, h3 in ((t0, h3a), (t0 + 1, h3b)):
                            x3 = mpool.tile([128, CH], BF16,
                                            tag=f"x3{ti % 2}")
                            nc.scalar.activation(
                                x3[:].rearrange("p (h c) -> p h c", h=2),
                                h3[:, :, 0:HCOL], AF.Prelu,
                                bias=b12t[:, 1:2], alpha=LEAKY)
                            for b in range(CBLK):
                                nc.tensor.matmul(
                                    dtc[:, b, ti, :],
                                    lhsT=x3[:, b * 128:(b + 1) * 128],
                                    rhs=w3t[:], start=True, stop=True)
                    front_of[c] = (dtc, dh)

                def do_corr(c):
                    dtc, dh = front_of.pop(c)
                    c0t = ctpool.tile([128, CBLK, 12], F32, tag="c0t")
                    c3t = ctpool.tile([128, CBLK, 12], F32, tag="c3t")
                    s0 = geopool.tile([128, CBLK, 4], F32, tag="s0")
                    s3 = geopool.tile([128, CBLK, 4], F32, tag="s3")
                    nc.vector.tensor_scalar(
                        s0[:], dtc[:, :, :, 0], scalar1=negh[:],
                        scalar2=b3t[:, 0:1],
                        op0=mybir.AluOpType.mult, op1=mybir.AluOpType.add)
                    nc.vector.tensor_scalar(
                        s3[:], dtc[:, :, :, 1], scalar1=posh[:],
                        scalar2=b3t[:, 1:2],
                        op0=mybir.AluOpType.mult, op1=mybir.AluOpType.add)
                    for x in range(3):
                        nc.vector.tensor_mul(c0t[:, :, x::3], dh[:, :, x::3],
                                             s0[:])
                        nc.vector.tensor_mul(c3t[:, :, x::3], dh[:, :, x::3],
                                             s3[:])
                    ctof[c] = (c0t, c3t)

                def do_scatter(c):
                    c0t, c3t = ctof.pop(c)
                    nc.gpsimd.dma_scatter_add(
                        A0[:, :12], c0t[:],
                        sixt[:, c * GC:(c + 1) * GC],
                        CH, CH, 12, elem_step=A_COLS,
                        queue_num=next_q())
                    nc.gpsimd.dma_scatter_add(
                        A3[:, :12], c3t[:],
                        sixt[:, GI + c * GC:GI + (c + 1) * GC],
                        CH, CH, 12, elem_step=A_COLS,
                        queue_num=next_q())
                    del Gof[c]

                # pipeline: gathers 1 ahead; front(c) overlaps mlp(c-1) on
                # DVE/PE; corrections+scatter of c-1 emitted before mlp(c)
                do_gather(0)
                do_front(0)
                for c in range(NCHUNK):
                    if c + 1 < NCHUNK:
                        do_gather(c + 1)
                    do_mlp(c)
                    if c + 1 < NCHUNK:
                        do_front(c + 1)
                    do_corr(c)
                    do_scatter(c)

    nc.compile()
    return nc


def _get_compiled():
    global _compiled
    if _compiled is None:
        _compiled = _build()
    return _compiled


# ------------------------------ entry point -------------------------------

def _prep_in_maps(coords, propers, encoded, t, answer, W0, b0, W1, b1, W2, b2,
                  W3, b3):
    coords = np.asarray(coords, dtype=np.float32)
    propers_np = np.asarray(propers)
    encoded = np.asarray(encoded, dtype=np.float32)
    t = np.asarray(t, dtype=np.float32)
    W0 = np.asarray(W0, dtype=np.float32)
    b0 = np.asarray(b0, dtype=np.float32)
    W1 = np.asarray(W1, dtype=np.float32)
    b1 = np.asarray(b1, dtype=np.float32)
    W2 = np.asarray(W2, dtype=np.float32)
    b2 = np.asarray(b2, dtype=np.float32)
    W3 = np.asarray(W3, dtype=np.float32)
    b3 = np.asarray(b3, dtype=np.float32)

    encTa = np.zeros((D, NA), dtype=_BF16)
    encTa[:, :N_ATOMS] = encoded.T.astype(_BF16)
    cflat = np.zeros((NA, 12), dtype=np.float32)
    cflat[:N_ATOMS] = coords.reshape(N_ATOMS, 12)
    coordsb = cflat.view(np.uint16).view(_BF16)

    w0all = np.concatenate([W0[128 * k:128 * (k + 1)] for k in range(4)],
                           axis=1).astype(_BF16)
    # wg4[4t+f, ti, :] selects time ti's geo features only
    wg4 = np.zeros((16, 4, 128), dtype=_BF16)
    for ti in range(T_STEPS):
        wg4[4 * ti + 0, ti] = W0[513].astype(_BF16)   # sin
        wg4[4 * ti + 1, ti] = W0[514].astype(_BF16)   # cos
        wg4[4 * ti + 2, ti] = W0[515].astype(_BF16)   # dl
    bx1 = np.zeros((D, 4), dtype=np.float32)
    for ti in range(T_STEPS):
        bx1[:, ti] = b0 + t[ti] * W0[512]
    bias12 = np.stack([b1, b2], axis=1).astype(np.float32)
    b3h = np.zeros((D, 2), dtype=np.float32)
    b3h[:, 0] = -0.5 * b3[0]
    b3h[:, 1] = 0.5 * b3[1]

    shared = {
        "encT": encTa,
        "coordsb": np.ascontiguousarray(coordsb),
        "w0all": w0all,
        "wg4": wg4,
        "w1": W1.astype(_BF16),
        "w2": W2.astype(_BF16),
        "w3": W3.astype(_BF16),
        "bx1": bx1,
        "bias12": bias12,
        "b3h": b3h,
    }

    props32 = propers_np.astype(np.int32)
    in_maps = []
    for cidx in range(N_CORES):
        shard = np.zeros((PPCT, 4), dtype=np.int32)
        shard[:PPC] = props32[cidx * PPC:(cidx + 1) * PPC]
        order = _order_props(shard, PPC, seed=cidx)
        po = shard[order]
        is_pad = order >= PPC
        gi = np.concatenate([_wrap_idxs(po[:, k]) for k in range(4)], axis=1)
        tgt0 = np.where(is_pad, DUMP, po[:, 0]).astype(np.int32)
        tgt3 = np.where(is_pad, DUMP, po[:, 3]).astype(np.int32)
        si = np.concatenate([_wrap_idxs(tgt0), _wrap_idxs(tgt3)], axis=1)
        in_maps.append({**shared, "gidx": gi, "sidx": si})
    return in_maps


def kernel(coords, propers, encoded, t, answer, W0, b0, W1, b1, W2, b2, W3, b3,
           _trace=False):
    from concourse.bass_utils import run_bass_kernel_spmd

    answer = np.asarray(answer, dtype=np.float32)
    in_maps = _prep_in_maps(coords, propers, encoded, t, answer, W0, b0, W1,
                            b1, W2, b2, W3, b3)
    nc = _get_compiled()
    res = run_bass_kernel_spmd(nc, in_maps, core_ids=list(range(N_CORES)),
                               trace=_trace)
    if _trace:
        kernel.last_exec_ns = res.exec_time_ns
        kernel.last_results = res

    acc = np.zeros((N_ATOMS, 12), dtype=np.float32)
    for cidx in range(N_CORES):
        acc += res.results[cidx]["A0"][:N_ATOMS, :12]
        acc += res.results[cidx]["A3"][:N_ATOMS, :12]
    out = answer + acc.reshape(N_ATOMS, T_STEPS, 3)
    return out.astype(np.float32)


kernel.last_exec_ns = None
kernel.last_results = None


# revision 44
# speedup vs baseline: 1.2306x; 1.2306x over previous
"""Trainium2 Bass kernel for DiffusionPropers (gnn_message_passing), v6.

No device-side table build at all: both gather tables are host inputs.
  - enc^T arrives per chunk via 4 SWDGE transpose-mode gathers straight
    from an HBM [NA, 256B] bf16 table (one 256B descriptor per torsion
    endpoint), giving the [128, props] layout the transposed MLP wants.
  - coords arrive per chunk via 4 non-transposed gathers from an HBM
    [NA, 64] f32 table (256B descriptors) in props layout.
  - Z = sum_k W0_k^T enc_k^T is 8 accumulating matmuls per chunk.
  - all 10 ring transfers per chunk rotate round-robin over the 4 SWDGE
    queues; scatter-add as before.
  - MLP: h1 = identity-broadcast(Z) + masked-wg @ geoT, per-t bias via
    the activation; ti-pair emission so PE/Act overlap; front/mlp/corr
    stage split so DVE geometry of chunk c+1 overlaps MLP of chunk c.
"""
import numpy as np
import ml_dtypes

# ---------------- compile-time constants (hardcoded problem shape) --------
N_ATOMS = 25000
NA = 25088              # padded atoms (196 * 128)
P_TOT = 100000
T_STEPS = 4
D = 128
N_CORES = 8
PPC = 12500             # real props per core
PPCT = 12544            # padded props per core (98 tiles of 128)
NTILES = PPCT // 128    # 98
CH = 896                # props per gather/scatter call (SWDGE ring limit)
NCHUNK = PPCT // CH     # 14
CBLK = CH // 128        # 7
HCOL = CH // 2          # 448 cols per PSUM-bank half
SLAB = 256              # bf16 elems per table slab (512B)
DUMP = NA               # scatter dump row
A_ROWS = NA + 8         # accumulator rows (incl. dump)
A_COLS = 64             # 256B stride for scatter
LEAKY = 0.001

_BF16 = ml_dtypes.bfloat16

_compiled = None


# ------------------------- host-side helpers ------------------------------

def _wrap_idxs(idx: np.ndarray) -> np.ndarray:
    """[n] int -> [128, n/16] int16, wrapped in 16 partitions, replicated x8."""
    n = idx.shape[0]
    assert n % 16 == 0
    w = idx.reshape(-1, 16).T.astype(np.int16)
    return np.tile(w, (8, 1))


def _order_props(props: np.ndarray, n_real: int, seed: int = 0) -> np.ndarray:
    """Order PPCT props so that within every aligned CH-chunk the p0 targets
    are distinct and the p3 targets are distinct (scatter-add race freedom)."""
    n = props.shape[0]
    rng = np.random.default_rng(seed)
    for attempt in range(50):
        perm = rng.permutation(n_real)
        buckets: list[list[int]] = [[] for _ in range(NCHUNK)]
        used0: list[set] = [set() for _ in range(NCHUNK)]
        used3: list[set] = [set() for _ in range(NCHUNK)]
        fail = []
        start = 0
        for j in perm:
            a0 = int(props[j, 0])
            a3 = int(props[j, 3])
            for d in range(NCHUNK):
                b = (start + d) % NCHUNK
                if (len(buckets[b]) < CH and a0 not in used0[b]
                        and a3 not in used3[b]):
                    buckets[b].append(int(j))
                    used0[b].add(a0)
                    used3[b].add(a3)
                    break
            else:
                fail.append(int(j))
            start = (start + 1) % NCHUNK
        if fail:
            continue
        pads = list(range(n_real, n))
        for b in range(NCHUNK):
            while len(buckets[b]) < CH:
                buckets[b].append(pads.pop())
        assert not pads
        order = [j for b in buckets for j in b]
        return np.array(order, dtype=np.int64)
    raise RuntimeError("prop ordering failed")


# ------------------------- device kernel build ----------------------------

def _build():
    import concourse.bass as bass
    import concourse.bacc as bacc
    import concourse.mybir as mybir
    import concourse.tile as tile
    from concourse.masks import make_identity
    from concourse.library_config import mlp as mlp_lib

    F32 = mybir.dt.float32
    BF16 = mybir.dt.bfloat16
    I16 = mybir.dt.int16
    AF = mybir.ActivationFunctionType

    nc = bacc.Bacc("TRN2", target_bir_lowering=False, debug=False,
                   num_devices=N_CORES, num_swdge_queues=4)

    # ---- I/O ----
    eslab = nc.dram_tensor("eslab", [NA, 128], BF16, kind="ExternalInput")
    cslab = nc.dram_tensor("cslab", [NA, 64], F32, kind="ExternalInput")
    w0all = nc.dram_tensor("w0all", [D, 4, 128], BF16, kind="ExternalInput")
    wg4 = nc.dram_tensor("wg4", [16, 4, 128], BF16, kind="ExternalInput")
    w1 = nc.dram_tensor("w1", [D, D], BF16, kind="ExternalInput")
    w2 = nc.dram_tensor("w2", [D, D], BF16, kind="ExternalInput")
    w3 = nc.dram_tensor("w3", [D, 2], BF16, kind="ExternalInput")
    bx1 = nc.dram_tensor("bx1", [D, 4], F32, kind="ExternalInput")
    bias12 = nc.dram_tensor("bias12", [D, 2], F32, kind="ExternalInput")
    b3h = nc.dram_tensor("b3h", [D, 2], F32, kind="ExternalInput")
    gidx = nc.dram_tensor("gidx", [128, 4 * (PPCT // 16)], I16,
                          kind="ExternalInput")
    sidx = nc.dram_tensor("sidx", [128, 2 * (PPCT // 16)], I16,
                          kind="ExternalInput")
    A0 = nc.dram_tensor("A0", [A_ROWS, A_COLS], F32, kind="ExternalOutput")
    A3 = nc.dram_tensor("A3", [A_ROWS, A_COLS], F32, kind="ExternalOutput")

    GI = PPCT // 16     # 784
    GC = CH // 16       # 56

    with tile.TileContext(nc) as tc:
        with (
            tc.tile_pool(name="const", bufs=1) as cpool,
        ):
            nc.gpsimd.load_library(mlp_lib)

            # ---- constants ----
            ibf = cpool.tile([128, 128], BF16)
            make_identity(nc, ibf[:])
            if32 = cpool.tile([128, 128], F32)
            make_identity(nc, if32[:])
            zero_b = cpool.tile([128, 1], F32)
            nc.vector.memset(zero_b[:], 0.0)
            eps_b = cpool.tile([128, 1], F32)
            nc.vector.memset(eps_b[:], 1e-12)
            negh = cpool.tile([128, 1], F32)
            nc.vector.memset(negh[:], -0.5)
            posh = cpool.tile([128, 1], F32)
            nc.vector.memset(posh[:], 0.5)

            w0t = cpool.tile([D, 4, 128], BF16)
            nc.sync.dma_start(out=w0t[:], in_=w0all[:])
            wgt = cpool.tile([16, 4, 128], BF16)
            nc.sync.dma_start(out=wgt[:], in_=wg4[:])
            w1t = cpool.tile([D, D], BF16)
            nc.sync.dma_start(out=w1t[:], in_=w1[:])
            w2t = cpool.tile([D, D], BF16)
            nc.sync.dma_start(out=w2t[:], in_=w2[:])
            w3t = cpool.tile([D, 2], BF16)
            nc.sync.dma_start(out=w3t[:], in_=w3[:])
            bx1t = cpool.tile([D, 4], F32)
            nc.sync.dma_start(out=bx1t[:], in_=bx1[:])
            b12t = cpool.tile([D, 2], F32)
            nc.sync.dma_start(out=b12t[:], in_=bias12[:])
            b3t = cpool.tile([D, 2], F32)
            nc.sync.dma_start(out=b3t[:], in_=b3h[:])
            gixt = cpool.tile([128, 4 * GI], I16)
            nc.sync.dma_start(out=gixt[:], in_=gidx[:])
            sixt = cpool.tile([128, 2 * GI], I16)
            nc.sync.dma_start(out=sixt[:], in_=sidx[:])

            # ================= Phase 1: main loop =================
            with (
                tc.tile_pool(name="gat", bufs=3) as gpool,
                tc.tile_pool(name="mlp", bufs=2) as mpool,
                tc.tile_pool(name="geo", bufs=2) as geopool,
                tc.tile_pool(name="cto", bufs=3) as ctpool,
                tc.tile_pool(name="hps", bufs=2, space="PSUM") as hps,
                tc.tile_pool(name="msc", bufs=2, space="PSUM") as mscps,
                tc.tile_pool(name="dt", bufs=1, space="PSUM") as dtps,
            ):
                Gof = {}
                ctof = {}
                qctr = [0]

                def next_q():
                    q = qctr[0] % 4
                    qctr[0] += 1
                    return q

                def do_gather(c):
                    E, C = [], []
                    for k in range(4):
                        ge = gpool.tile([128, 1, CH], BF16, tag=f"ge{k}")
                        nc.gpsimd.dma_gather(
                            ge[:], eslab[:],
                            gixt[:, k * GI + c * GC:
                                 k * GI + (c + 1) * GC],
                            CH, CH, 128, transpose=True,
                            queue_num=next_q())
                        E.append(ge)
                        gc = gpool.tile([128, CBLK, 64], F32, tag=f"gc{k}")
                        nc.gpsimd.dma_gather(
                            gc[:], cslab[:],
                            gixt[:, k * GI + c * GC:
                                 k * GI + (c + 1) * GC],
                            CH, CH, 64, queue_num=next_q())
                        C.append(gc)
                    Gof[c] = (E, C)

                front_of = {}

                def do_front(c):
                    E, C = Gof[c]
                    cco = [C[k][:, :, 0:12] for k in range(4)]

                    # ---- Z^T = sum_k W0_k^T enc_k^T (8 matmuls) ----
                    zbf = mpool.tile([128, CH], BF16, tag="zbf")
                    for h in range(2):
                        sl = slice(h * HCOL, (h + 1) * HCOL)
                        zps = mscps.tile([128, 512], F32, tag="msc")
                        for k in range(4):
                            nc.tensor.matmul(
                                zps[:, 0:HCOL],
                                lhsT=w0t[:, k, :], rhs=E[k][:, 0, sl],
                                start=(k == 0), stop=(k == 3))
                        nc.vector.tensor_copy(zbf[:, sl], zps[:, 0:HCOL])

                    # ---- geometry (props layout) ----
                    u1 = geopool.tile([128, CBLK, 12], F32, tag="u1")
                    u2 = geopool.tile([128, CBLK, 12], F32, tag="u2")
                    u3 = geopool.tile([128, CBLK, 12], F32, tag="u3")
                    dr = geopool.tile([128, CBLK, 12], F32, tag="dr")
                    nc.vector.tensor_sub(u1[:], cco[1], cco[0])
                    nc.vector.tensor_sub(u2[:], cco[2], cco[1])
                    nc.vector.tensor_sub(u3[:], cco[3], cco[2])
                    nc.vector.tensor_sub(dr[:], cco[0], cco[3])

                    def cross(out, a, b):
                        tmp = geopool.tile([128, CBLK, 4], F32, tag="ctmp")
                        for x in range(3):
                            y, z = (x + 1) % 3, (x + 2) % 3
                            nc.vector.tensor_mul(tmp[:], a[:, :, y::3],
                                                 b[:, :, z::3])
                            nc.vector.tensor_mul(out[:, :, x::3],
                                                 a[:, :, z::3], b[:, :, y::3])
                            nc.vector.tensor_sub(out[:, :, x::3], tmp[:],
                                                 out[:, :, x::3])

                    cr12 = geopool.tile([128, CBLK, 12], F32, tag="cr12")
                    cr23 = geopool.tile([128, CBLK, 12], F32, tag="cr23")
                    cross(cr12, u1, u2)
                    cross(cr23, u2, u3)

                    def dot3(out, a, b, tmp):
                        nc.vector.tensor_mul(tmp[:], a[:], b[:])
                        nc.vector.tensor_add(out[:], tmp[:, :, 0::3],
                                             tmp[:, :, 1::3])
                        nc.vector.tensor_add(out[:], out[:], tmp[:, :, 2::3])

                    tmp12 = geopool.tile([128, CBLK, 12], F32, tag="tmp12")
                    n2 = geopool.tile([128, CBLK, 4], F32, tag="n2")
                    dot3(n2, u2, u2, tmp12)
                    nc.scalar.activation(n2[:], n2[:], AF.Sqrt, bias=zero_b[:])
                    sn = geopool.tile([128, CBLK, 4], F32, tag="sn")
                    dot3(sn, u1, cr23, tmp12)
                    nc.vector.tensor_mul(sn[:], sn[:], n2[:])
                    cn = geopool.tile([128, CBLK, 4], F32, tag="cn")
                    dot3(cn, cr12, cr23, tmp12)
                    hy = geopool.tile([128, CBLK, 4], F32, tag="hy")
                    t2 = geopool.tile([128, CBLK, 4], F32, tag="t2")
                    nc.vector.tensor_mul(hy[:], sn[:], sn[:])
                    nc.vector.tensor_mul(t2[:], cn[:], cn[:])
                    nc.vector.tensor_add(hy[:], hy[:], t2[:])
                    nc.scalar.activation(hy[:], hy[:], AF.Sqrt, bias=eps_b[:])
                    rh = geopool.tile([128, CBLK, 4], F32, tag="rh")
                    nc.vector.reciprocal(rh[:], hy[:])
                    dl = geopool.tile([128, CBLK, 4], F32, tag="dl")
                    dot3(dl, dr, dr, tmp12)
                    nc.scalar.activation(dl[:], dl[:], AF.Sqrt, bias=eps_b[:])
                    rdl = geopool.tile([128, CBLK, 4], F32, tag="rdl")
                    nc.vector.reciprocal(rdl[:], dl[:])
                    dh = geopool.tile([128, CBLK, 12], F32, tag="dh")
                    for x in range(3):
                        nc.vector.tensor_mul(dh[:, :, x::3], dr[:, :, x::3],
                                             rdl[:])
                    # geo features per (prop, t): [sin, cos, dl, pad]
                    geo = geopool.tile([128, CBLK, 16], F32, tag="geo")
                    nc.vector.memset(geo[:, :, 3::4], 0.0)
                    nc.vector.tensor_mul(geo[:, :, 0::4], sn[:], rh[:])
                    nc.vector.tensor_mul(geo[:, :, 1::4], cn[:], rh[:])
                    nc.vector.tensor_copy(geo[:, :, 2::4], dl[:])

                    # geoT [16, 896]
                    geoT = mpool.tile([16, CH], BF16, tag="geoT")
                    for hb, nb in ((0, 4), (4, 3)):
                        gtp = mscps.tile([128, 512], F32, tag="msc")
                        for b in range(nb):
                            nc.tensor.matmul(
                                gtp[0:16, b * 128:(b + 1) * 128],
                                lhsT=geo[:, hb + b, :],
                                rhs=if32[:], is_transpose=True,
                                start=True, stop=True)
                        nc.vector.tensor_copy(
                            geoT[:, hb * 128:(hb + nb) * 128],
                            gtp[0:16, 0:nb * 128])

                    front_of[c] = (zbf, geoT, dh)

                def do_mlp(c):
                    zbf, geoT, dh = front_of[c]

                    # ---- per-t MLP (ti pairs, layer-major in pair) ----
                    dtc = dtps.tile([128, CBLK, 4, 2], F32, tag="dtc")

                    def mm_h1(ti):
                        h1 = hps.tile([128, 2, 512], F32, tag="h")
                        for h in range(2):
                            sl = slice(h * HCOL, (h + 1) * HCOL)
                            nc.tensor.matmul(
                                h1[:, h, 0:HCOL], lhsT=ibf[:],
                                rhs=zbf[:, sl], start=True, stop=False)
                            nc.tensor.matmul(
                                h1[:, h, 0:HCOL],
                                lhsT=wgt[:, ti, :],
                                rhs=geoT[:, sl],
                                start=False, stop=True)
                        return h1

                    def mm_layer(w, x):
                        hp = hps.tile([128, 2, 512], F32, tag="h")
                        for h in range(2):
                            nc.tensor.matmul(hp[:, h, 0:HCOL], lhsT=w[:],
                                             rhs=x[:, h, :],
                                             start=True, stop=True)
                        return hp

                    def act(hp, bias, tag):
                        x = mpool.tile([128, 2, HCOL], BF16, tag=tag)
                        nc.scalar.activation(x[:], hp[:, :, 0:HCOL],
                                             AF.Prelu, bias=bias, alpha=LEAKY)
                        return x

                    for t0 in (0, 2):
                        h1a = mm_h1(t0)
                        h1b = mm_h1(t0 + 1)
                        x1a = act(h1a, bx1t[:, t0:t0 + 1], "x1a")
                        x1b = act(h1b, bx1t[:, t0 + 1:t0 + 2], "x1b")
                        h2a = mm_layer(w1t, x1a)
                        h2b = mm_layer(w1t, x1b)
                        x2a = act(h2a, b12t[:, 0:1], "x2a")
                        x2b = act(h2b, b12t[:, 0:1], "x2b")
                        h3a = mm_layer(w2t, x2a)
                        h3b = mm_layer(w2t, x2b)
                        for ti, h3 in ((t0, h3a), (t0 + 1, h3b)):
                            x3 = mpool.tile([128, CH], BF16,
                                            tag=f"x3{ti % 2}")
                            nc.scalar.activation(
                                x3[:].rearrange("p (h c) -> p h c", h=2),
                                h3[:, :, 0:HCOL], AF.Prelu,
                                bias=b12t[:, 1:2], alpha=LEAKY)
                            for b in range(CBLK):
                                nc.tensor.matmul(
                                    dtc[:, b, ti, :],
                                    lhsT=x3[:, b * 128:(b + 1) * 128],
                                    rhs=w3t[:], start=True, stop=True)
                    front_of[c] = (dtc, dh)

                def do_corr(c):
                    dtc, dh = front_of.pop(c)
                    c0t = ctpool.tile([128, CBLK, 12], F32, tag="c0t")
                    c3t = ctpool.tile([128, CBLK, 12], F32, tag="c3t")
                    s0 = geopool.tile([128, CBLK, 4], F32, tag="s0")
                    s3 = geopool.tile([128, CBLK, 4], F32, tag="s3")
                    nc.vector.tensor_scalar(
                        s0[:], dtc[:, :, :, 0], scalar1=negh[:],
                        scalar2=b3t[:, 0:1],
                        op0=mybir.AluOpType.mult, op1=mybir.AluOpType.add)
                    nc.vector.tensor_scalar(
                        s3[:], dtc[:, :, :, 1], scalar1=posh[:],
                        scalar2=b3t[:, 1:2],
                        op0=mybir.AluOpType.mult, op1=mybir.AluOpType.add)
                    for x in range(3):
                        nc.vector.tensor_mul(c0t[:, :, x::3], dh[:, :, x::3],
                                             s0[:])
                        nc.vector.tensor_mul(c3t[:, :, x::3], dh[:, :, x::3],
                                             s3[:])
                    ctof[c] = (c0t, c3t)

                def do_scatter(c):
                    c0t, c3t = ctof.pop(c)
                    nc.gpsimd.dma_scatter_add(
                        A0[:, :12], c0t[:],
                        sixt[:, c * GC:(c + 1) * GC],
                        CH, CH, 12, elem_step=A_COLS,
                        queue_num=next_q())
                    nc.gpsimd.dma_scatter_add(
                        A3[:, :12], c3t[:],
                        sixt[:, GI + c * GC:GI + (c + 1) * GC],
                        CH, CH, 12, elem_step=A_COLS,
                        queue_num=next_q())
                    del Gof[c]

                # pipeline: gathers 1 ahead; front(c) overlaps mlp(c-1) on
                # DVE/PE; corrections+scatter of c-1 emitted before mlp(c)
                do_gather(0)
                do_front(0)
                for c in range(NCHUNK):
                    if c + 1 < NCHUNK:
                        do_gather(c + 1)
                    do_mlp(c)
                    if c + 1 < NCHUNK:
                        do_front(c + 1)
                    do_corr(c)
                    do_scatter(c)

    nc.compile()
    return nc


def _get_compiled():
    global _compiled
    if _compiled is None:
        _compiled = _build()
    return _compiled


# ------------------------------ entry point -------------------------------

def _prep_in_maps(coords, propers, encoded, t, answer, W0, b0, W1, b1, W2, b2,
                  W3, b3):
    coords = np.asarray(coords, dtype=np.float32)
    propers_np = np.asarray(propers)
    encoded = np.asarray(encoded, dtype=np.float32)
    t = np.asarray(t, dtype=np.float32)
    W0 = np.asarray(W0, dtype=np.float32)
    b0 = np.asarray(b0, dtype=np.float32)
    W1 = np.asarray(W1, dtype=np.float32)
    b1 = np.asarray(b1, dtype=np.float32)
    W2 = np.asarray(W2, dtype=np.float32)
    b2 = np.asarray(b2, dtype=np.float32)
    W3 = np.asarray(W3, dtype=np.float32)
    b3 = np.asarray(b3, dtype=np.float32)

    eslab = np.zeros((NA, 128), dtype=_BF16)
    eslab[:N_ATOMS] = encoded.astype(_BF16)
    cslab = np.zeros((NA, 64), dtype=np.float32)
    cslab[:N_ATOMS, 0:12] = coords.reshape(N_ATOMS, 12)

    w0all = np.stack([W0[128 * k:128 * (k + 1)] for k in range(4)],
                     axis=1).astype(_BF16)
    # wg4[4t+f, ti, :] selects time ti's geo features only
    wg4 = np.zeros((16, 4, 128), dtype=_BF16)
    for ti in range(T_STEPS):
        wg4[4 * ti + 0, ti] = W0[513].astype(_BF16)   # sin
        wg4[4 * ti + 1, ti] = W0[514].astype(_BF16)   # cos
        wg4[4 * ti + 2, ti] = W0[515].astype(_BF16)   # dl
    bx1 = np.zeros((D, 4), dtype=np.float32)
    for ti in range(T_STEPS):
        bx1[:, ti] = b0 + t[ti] * W0[512]
    bias12 = np.stack([b1, b2], axis=1).astype(np.float32)
    b3h = np.zeros((D, 2), dtype=np.float32)
    b3h[:, 0] = -0.5 * b3[0]
    b3h[:, 1] = 0.5 * b3[1]

    shared = {
        "eslab": eslab,
        "cslab": cslab,
        "w0all": w0all,
        "wg4": wg4,
        "w1": W1.astype(_BF16),
        "w2": W2.astype(_BF16),
        "w3": W3.astype(_BF16),
        "bx1": bx1,
        "bias12": bias12,
        "b3h": b3h,
    }

    props32 = propers_np.astype(np.int32)
    in_maps = []
    for cidx in range(N_CORES):
        shard = np.zeros((PPCT, 4), dtype=np.int32)
        shard[:PPC] = props32[cidx * PPC:(cidx + 1) * PPC]
        order = _order_props(shard, PPC, seed=cidx)
        po = shard[order]
        is_pad = order >= PPC
        gi = np.concatenate([_wrap_idxs(po[:, k]) for k in range(4)], axis=1)
        tgt0 = np.where(is_pad, DUMP, po[:, 0]).astype(np.int32)
        tgt3 = np.where(is_pad, DUMP, po[:, 3]).astype(np.int32)
        si = np.concatenate([_wrap_idxs(tgt0), _wrap_idxs(tgt3)], axis=1)
        in_maps.append({**shared, "gidx": gi, "sidx": si})
    return in_maps


def kernel(coords, propers, encoded, t, answer, W0, b0, W1, b1, W2, b2, W3, b3,
           _trace=False):
    from concourse.bass_utils import run_bass_kernel_spmd

    answer = np.asarray(answer, dtype=np.float32)
    in_maps = _prep_in_maps(coords, propers, encoded, t, answer, W0, b0, W1,
                            b1, W2, b2, W3, b3)
    nc = _get_compiled()
    res = run_bass_kernel_spmd(nc, in_maps, core_ids=list(range(N_CORES)),
                               trace=_trace)
    if _trace:
        kernel.last_exec_ns = res.exec_time_ns
        kernel.last_results = res

    acc = np.zeros((N_ATOMS, 12), dtype=np.float32)
    for cidx in range(N_CORES):
        acc += res.results[cidx]["A0"][:N_ATOMS, :12]
        acc += res.results[cidx]["A3"][:N_ATOMS, :12]
    out = answer + acc.reshape(N_ATOMS, T_STEPS, 3)
    return out.astype(np.float32)


kernel.last_exec_ns = None
kernel.last_results = None


# revision 46
# speedup vs baseline: 1.2321x; 1.0012x over previous
"""Trainium2 Bass kernel for DiffusionPropers (gnn_message_passing), v6.

No device-side table build at all: both gather tables are host inputs.
  - enc^T arrives per chunk via 4 SWDGE transpose-mode gathers straight
    from an HBM [NA, 256B] bf16 table (one 256B descriptor per torsion
    endpoint), giving the [128, props] layout the transposed MLP wants.
  - coords arrive per chunk via 4 non-transposed gathers from an HBM
    [NA, 64] f32 table (256B descriptors) in props layout.
  - Z = sum_k W0_k^T enc_k^T is 8 accumulating matmuls per chunk.
  - all 10 ring transfers per chunk rotate round-robin over the 4 SWDGE
    queues; scatter-add as before.
  - MLP: h1 = identity-broadcast(Z) + masked-wg @ geoT, per-t bias via
    the activation; ti-pair emission so PE/Act overlap; front/mlp/corr
    stage split so DVE geometry of chunk c+1 overlaps MLP of chunk c.
"""
import numpy as np
import ml_dtypes

# ---------------- compile-time constants (hardcoded problem shape) --------
N_ATOMS = 25000
NA = 25088              # padded atoms (196 * 128)
P_TOT = 100000
T_STEPS = 4
D = 128
N_CORES = 8
PPC = 12500             # real props per core
PPCT = 12544            # padded props per core (98 tiles of 128)
NTILES = PPCT // 128    # 98
CH = 896                # props per gather/scatter call (SWDGE ring limit)
NCHUNK = PPCT // CH     # 14
CBLK = CH // 128        # 7
HCOL = CH // 2          # 448 cols per PSUM-bank half
SLAB = 256              # bf16 elems per table slab (512B)
DUMP = NA               # scatter dump row
A_ROWS = NA + 8         # accumulator rows (incl. dump)
A_COLS = 64             # 256B stride for scatter
LEAKY = 0.001

_BF16 = ml_dtypes.bfloat16

_compiled = None


# ------------------------- host-side helpers ------------------------------

def _wrap_idxs(idx: np.ndarray) -> np.ndarray:
    """[n] int -> [128, n/16] int16, wrapped in 16 partitions, replicated x8."""
    n = idx.shape[0]
    assert n % 16 == 0
    w = idx.reshape(-1, 16).T.astype(np.int16)
    return np.tile(w, (8, 1))


def _order_props(props: np.ndarray, n_real: int, seed: int = 0) -> np.ndarray:
    """Order PPCT props so that within every aligned CH-chunk the p0 targets
    are distinct and the p3 targets are distinct (scatter-add race freedom)."""
    n = props.shape[0]
    rng = np.random.default_rng(seed)
    for attempt in range(50):
        perm = rng.permutation(n_real)
        buckets: list[list[int]] = [[] for _ in range(NCHUNK)]
        used0: list[set] = [set() for _ in range(NCHUNK)]
        used3: list[set] = [set() for _ in range(NCHUNK)]
        fail = []
        start = 0
        for j in perm:
            a0 = int(props[j, 0])
            a3 = int(props[j, 3])
            for d in range(NCHUNK):
                b = (start + d) % NCHUNK
                if (len(buckets[b]) < CH and a0 not in used0[b]
                        and a3 not in used3[b]):
                    buckets[b].append(int(j))
                    used0[b].add(a0)
                    used3[b].add(a3)
                    break
            else:
                fail.append(int(j))
            start = (start + 1) % NCHUNK
        if fail:
            continue
        pads = list(range(n_real, n))
        for b in range(NCHUNK):
            while len(buckets[b]) < CH:
                buckets[b].append(pads.pop())
        assert not pads
        order = [j for b in buckets for j in b]
        return np.array(order, dtype=np.int64)
    raise RuntimeError("prop ordering failed")


# ------------------------- device kernel build ----------------------------

def _build():
    import concourse.bass as bass
    import concourse.bacc as bacc
    import concourse.mybir as mybir
    import concourse.tile as tile
    from concourse.masks import make_identity
    from concourse.library_config import mlp as mlp_lib

    F32 = mybir.dt.float32
    BF16 = mybir.dt.bfloat16
    I16 = mybir.dt.int16
    AF = mybir.ActivationFunctionType

    nc = bacc.Bacc("TRN2", target_bir_lowering=False, debug=False,
                   num_devices=N_CORES, num_swdge_queues=4)

    # ---- I/O ----
    eslab = nc.dram_tensor("eslab", [NA, 128], BF16, kind="ExternalInput")
    cslab = nc.dram_tensor("cslab", [NA, 64], F32, kind="ExternalInput")
    w0all = nc.dram_tensor("w0all", [D, 4, 128], BF16, kind="ExternalInput")
    wg4 = nc.dram_tensor("wg4", [16, 4, 128], BF16, kind="ExternalInput")
    w1 = nc.dram_tensor("w1", [D, D], BF16, kind="ExternalInput")
    w2 = nc.dram_tensor("w2", [D, D], BF16, kind="ExternalInput")
    w3 = nc.dram_tensor("w3", [D, 2], BF16, kind="ExternalInput")
    bx1 = nc.dram_tensor("bx1", [D, 4], F32, kind="ExternalInput")
    bias12 = nc.dram_tensor("bias12", [D, 2], F32, kind="ExternalInput")
    b3h = nc.dram_tensor("b3h", [D, 2], F32, kind="ExternalInput")
    gidx = nc.dram_tensor("gidx", [128, 4 * (PPCT // 16)], I16,
                          kind="ExternalInput")
    sidx = nc.dram_tensor("sidx", [128, 2 * (PPCT // 16)], I16,
                          kind="ExternalInput")
    A0 = nc.dram_tensor("A0", [A_ROWS, A_COLS], F32, kind="ExternalOutput")
    A3 = nc.dram_tensor("A3", [A_ROWS, A_COLS], F32, kind="ExternalOutput")

    GI = PPCT // 16     # 784
    GC = CH // 16       # 56

    with tile.TileContext(nc) as tc:
        with (
            tc.tile_pool(name="const", bufs=1) as cpool,
        ):
            nc.gpsimd.load_library(mlp_lib)

            # ---- constants ----
            ibf = cpool.tile([128, 128], BF16)
            make_identity(nc, ibf[:])
            if32 = cpool.tile([128, 128], F32)
            make_identity(nc, if32[:])
            zero_b = cpool.tile([128, 1], F32)
            nc.vector.memset(zero_b[:], 0.0)
            eps_b = cpool.tile([128, 1], F32)
            nc.vector.memset(eps_b[:], 1e-12)
            negh = cpool.tile([128, 1], F32)
            nc.vector.memset(negh[:], -0.5)
            posh = cpool.tile([128, 1], F32)
            nc.vector.memset(posh[:], 0.5)

            w0t = cpool.tile([D, 4, 128], BF16)
            nc.sync.dma_start(out=w0t[:], in_=w0all[:])
            wgt = cpool.tile([16, 4, 128], BF16)
            nc.sync.dma_start(out=wgt[:], in_=wg4[:])
            w1t = cpool.tile([D, D], BF16)
            nc.sync.dma_start(out=w1t[:], in_=w1[:])
            w2t = cpool.tile([D, D], BF16)
            nc.sync.dma_start(out=w2t[:], in_=w2[:])
            w3t = cpool.tile([D, 2], BF16)
            nc.sync.dma_start(out=w3t[:], in_=w3[:])
            bx1t = cpool.tile([D, 4], F32)
            nc.sync.dma_start(out=bx1t[:], in_=bx1[:])
            b12t = cpool.tile([D, 2], F32)
            nc.sync.dma_start(out=b12t[:], in_=bias12[:])
            b3t = cpool.tile([D, 2], F32)
            nc.sync.dma_start(out=b3t[:], in_=b3h[:])
            gixt = cpool.tile([128, 4 * GI], I16)
            nc.sync.dma_start(out=gixt[:], in_=gidx[:])
            sixt = cpool.tile([128, 2 * GI], I16)
            nc.sync.dma_start(out=sixt[:], in_=sidx[:])

            # ================= Phase 1: main loop =================
            with (
                tc.tile_pool(name="gat", bufs=3) as gpool,
                tc.tile_pool(name="mlp", bufs=2) as mpool,
                tc.tile_pool(name="geo", bufs=2) as geopool,
                tc.tile_pool(name="cto", bufs=3) as ctpool,
                tc.tile_pool(name="hps", bufs=2, space="PSUM") as hps,
                tc.tile_pool(name="msc", bufs=2, space="PSUM") as mscps,
                tc.tile_pool(name="dt", bufs=1, space="PSUM") as dtps,
            ):
                Gof = {}
                ctof = {}
                qctr = [0]

                def next_q():
                    q = qctr[0] % 4
                    qctr[0] += 1
                    return q

                def do_gather(c, ks):
                    E, C = Gof.setdefault(c, ([], []))
                    for k in ks:
                        ge = gpool.tile([128, 1, CH], BF16, tag=f"ge{k}")
                        nc.gpsimd.dma_gather(
                            ge[:], eslab[:],
                            gixt[:, k * GI + c * GC:
                                 k * GI + (c + 1) * GC],
                            CH, CH, 128, transpose=True,
                            queue_num=next_q())
                        E.append(ge)
                        gc = gpool.tile([128, CBLK, 64], F32, tag=f"gc{k}")
                        nc.gpsimd.dma_gather(
                            gc[:], cslab[:],
                            gixt[:, k * GI + c * GC:
                                 k * GI + (c + 1) * GC],
                            CH, CH, 64, queue_num=next_q())
                        C.append(gc)

                front_of = {}

                def do_front(c):
                    E, C = Gof[c]
                    cco = [C[k][:, :, 0:12] for k in range(4)]

                    # ---- Z^T = sum_k W0_k^T enc_k^T (8 matmuls) ----
                    zbf = mpool.tile([128, CH], BF16, tag="zbf")
                    for h in range(2):
                        sl = slice(h * HCOL, (h + 1) * HCOL)
                        zps = mscps.tile([128, 512], F32, tag="msc")
                        for k in range(4):
                            nc.tensor.matmul(
                                zps[:, 0:HCOL],
                                lhsT=w0t[:, k, :], rhs=E[k][:, 0, sl],
                                start=(k == 0), stop=(k == 3))
                        nc.vector.tensor_copy(zbf[:, sl], zps[:, 0:HCOL])

                    # ---- geometry (props layout) ----
                    u1 = geopool.tile([128, CBLK, 12], F32, tag="u1")
                    u2 = geopool.tile([128, CBLK, 12], F32, tag="u2")
                    u3 = geopool.tile([128, CBLK, 12], F32, tag="u3")
                    dr = geopool.tile([128, CBLK, 12], F32, tag="dr")
                    nc.vector.tensor_sub(u1[:], cco[1], cco[0])
                    nc.vector.tensor_sub(u2[:], cco[2], cco[1])
                    nc.vector.tensor_sub(u3[:], cco[3], cco[2])
                    nc.vector.tensor_sub(dr[:], cco[0], cco[3])

                    def cross(out, a, b):
                        tmp = geopool.tile([128, CBLK, 4], F32, tag="ctmp")
                        for x in range(3):
                            y, z = (x + 1) % 3, (x + 2) % 3
                            nc.vector.tensor_mul(tmp[:], a[:, :, y::3],
                                                 b[:, :, z::3])
                            nc.vector.tensor_mul(out[:, :, x::3],
                                                 a[:, :, z::3], b[:, :, y::3])
                            nc.vector.tensor_sub(out[:, :, x::3], tmp[:],
                                                 out[:, :, x::3])

                    cr12 = geopool.tile([128, CBLK, 12], F32, tag="cr12")
                    cr23 = geopool.tile([128, CBLK, 12], F32, tag="cr23")
                    cross(cr12, u1, u2)
                    cross(cr23, u2, u3)

                    def dot3(out, a, b, tmp):
                        nc.vector.tensor_mul(tmp[:], a[:], b[:])
                        nc.vector.tensor_add(out[:], tmp[:, :, 0::3],
                                             tmp[:, :, 1::3])
                        nc.vector.tensor_add(out[:], out[:], tmp[:, :, 2::3])

                    tmp12 = geopool.tile([128, CBLK, 12], F32, tag="tmp12")
                    n2 = geopool.tile([128, CBLK, 4], F32, tag="n2")
                    dot3(n2, u2, u2, tmp12)
                    nc.scalar.activation(n2[:], n2[:], AF.Sqrt, bias=zero_b[:])
                    sn = geopool.tile([128, CBLK, 4], F32, tag="sn")
                    dot3(sn, u1, cr23, tmp12)
                    nc.vector.tensor_mul(sn[:], sn[:], n2[:])
                    cn = geopool.tile([128, CBLK, 4], F32, tag="cn")
                    dot3(cn, cr12, cr23, tmp12)
                    hy = geopool.tile([128, CBLK, 4], F32, tag="hy")
                    t2 = geopool.tile([128, CBLK, 4], F32, tag="t2")
                    nc.vector.tensor_mul(hy[:], sn[:], sn[:])
                    nc.vector.tensor_mul(t2[:], cn[:], cn[:])
                    nc.vector.tensor_add(hy[:], hy[:], t2[:])
                    nc.scalar.activation(hy[:], hy[:], AF.Sqrt, bias=eps_b[:])
                    rh = geopool.tile([128, CBLK, 4], F32, tag="rh")
                    nc.vector.reciprocal(rh[:], hy[:])
                    dl = geopool.tile([128, CBLK, 4], F32, tag="dl")
                    dot3(dl, dr, dr, tmp12)
                    nc.scalar.activation(dl[:], dl[:], AF.Sqrt, bias=eps_b[:])
                    rdl = geopool.tile([128, CBLK, 4], F32, tag="rdl")
                    nc.vector.reciprocal(rdl[:], dl[:])
                    dh = geopool.tile([128, CBLK, 12], F32, tag="dh")
                    for x in range(3):
                        nc.vector.tensor_mul(dh[:, :, x::3], dr[:, :, x::3],
                                             rdl[:])
                    # geo features per (prop, t): [sin, cos, dl, pad]
                    geo = geopool.tile([128, CBLK, 16], F32, tag="geo")
                    nc.vector.memset(geo[:, :, 3::4], 0.0)
                    nc.vector.tensor_mul(geo[:, :, 0::4], sn[:], rh[:])
                    nc.vector.tensor_mul(geo[:, :, 1::4], cn[:], rh[:])
                    nc.vector.tensor_copy(geo[:, :, 2::4], dl[:])

                    # geoT [16, 896]
                    geoT = mpool.tile([16, CH], BF16, tag="geoT")
                    for hb, nb in ((0, 4), (4, 3)):
                        gtp = mscps.tile([128, 512], F32, tag="msc")
                        for b in range(nb):
                            nc.tensor.matmul(
                                gtp[0:16, b * 128:(b + 1) * 128],
                                lhsT=geo[:, hb + b, :],
                                rhs=if32[:], is_transpose=True,
                                start=True, stop=True)
                        nc.vector.tensor_copy(
                            geoT[:, hb * 128:(hb + nb) * 128],
                            gtp[0:16, 0:nb * 128])

                    front_of[c] = (zbf, geoT, dh)

                def do_mlp(c):
                    zbf, geoT, dh = front_of[c]

                    # ---- per-t MLP (ti pairs, layer-major in pair) ----
                    dtc = dtps.tile([128, CBLK, 4, 2], F32, tag="dtc")

                    def mm_h1(ti):
                        h1 = hps.tile([128, 2, 512], F32, tag="h")
                        for h in range(2):
                            sl = slice(h * HCOL, (h + 1) * HCOL)
                            nc.tensor.matmul(
                                h1[:, h, 0:HCOL], lhsT=ibf[:],
                                rhs=zbf[:, sl], start=True, stop=False)
                            nc.tensor.matmul(
                                h1[:, h, 0:HCOL],
                                lhsT=wgt[:, ti, :],
                                rhs=geoT[:, sl],
                                start=False, stop=True)
                        return h1

                    def mm_layer(w, x):
                        hp = hps.tile([128, 2, 512], F32, tag="h")
                        for h in range(2):
                            nc.tensor.matmul(hp[:, h, 0:HCOL], lhsT=w[:],
                                             rhs=x[:, h, :],
                                             start=True, stop=True)
                        return hp

                    def act(hp, bias, tag):
                        x = mpool.tile([128, 2, HCOL], BF16, tag=tag)
                        nc.scalar.activation(x[:], hp[:, :, 0:HCOL],
                                             AF.Prelu, bias=bias, alpha=LEAKY)
                        return x

                    for t0 in (0, 2):
                        h1a = mm_h1(t0)
                        h1b = mm_h1(t0 + 1)
                        x1a = act(h1a, bx1t[:, t0:t0 + 1], "x1a")
                        x1b = act(h1b, bx1t[:, t0 + 1:t0 + 2], "x1b")
                        h2a = mm_layer(w1t, x1a)
                        h2b = mm_layer(w1t, x1b)
                        x2a = act(h2a, b12t[:, 0:1], "x2a")
                        x2b = act(h2b, b12t[:, 0:1], "x2b")
                        h3a = mm_layer(w2t, x2a)
                        h3b = mm_layer(w2t, x2b)
                        for ti, h3 in ((t0, h3a), (t0 + 1, h3b)):
                            x3 = mpool.tile([128, CH], BF16,
                                            tag=f"x3{ti % 2}")
                            nc.scalar.activation(
                                x3[:].rearrange("p (h c) -> p h c", h=2),
                                h3[:, :, 0:HCOL], AF.Prelu,
                                bias=b12t[:, 1:2], alpha=LEAKY)
                            for b in range(CBLK):
                                nc.tensor.matmul(
                                    dtc[:, b, ti, :],
                                    lhsT=x3[:, b * 128:(b + 1) * 128],
                                    rhs=w3t[:], start=True, stop=True)
                    front_of[c] = (dtc, dh)

                def do_corr(c):
                    dtc, dh = front_of.pop(c)
                    c0t = ctpool.tile([128, CBLK, 12], F32, tag="c0t")
                    c3t = ctpool.tile([128, CBLK, 12], F32, tag="c3t")
                    s0 = geopool.tile([128, CBLK, 4], F32, tag="s0")
                    s3 = geopool.tile([128, CBLK, 4], F32, tag="s3")
                    nc.vector.tensor_scalar(
                        s0[:], dtc[:, :, :, 0], scalar1=negh[:],
                        scalar2=b3t[:, 0:1],
                        op0=mybir.AluOpType.mult, op1=mybir.AluOpType.add)
                    nc.vector.tensor_scalar(
                        s3[:], dtc[:, :, :, 1], scalar1=posh[:],
                        scalar2=b3t[:, 1:2],
                        op0=mybir.AluOpType.mult, op1=mybir.AluOpType.add)
                    for x in range(3):
                        nc.vector.tensor_mul(c0t[:, :, x::3], dh[:, :, x::3],
                                             s0[:])
                        nc.vector.tensor_mul(c3t[:, :, x::3], dh[:, :, x::3],
                                             s3[:])
                    ctof[c] = (c0t, c3t)

                def do_scatter(c):
                    c0t, c3t = ctof.pop(c)
                    nc.gpsimd.dma_scatter_add(
                        A0[:, :12], c0t[:],
                        sixt[:, c * GC:(c + 1) * GC],
                        CH, CH, 12, elem_step=A_COLS,
                        queue_num=next_q())
                    nc.gpsimd.dma_scatter_add(
                        A3[:, :12], c3t[:],
                        sixt[:, GI + c * GC:GI + (c + 1) * GC],
                        CH, CH, 12, elem_step=A_COLS,
                        queue_num=next_q())
                    del Gof[c]

                # pipeline: gathers 1 ahead; front(c) overlaps mlp(c-1) on
                # DVE/PE; corrections+scatter of c-1 emitted before mlp(c)
                do_gather(0, (0, 1, 2, 3))
                do_front(0)
                for c in range(NCHUNK):
                    if c + 1 < NCHUNK:
                        do_gather(c + 1, (0, 1))
                    do_mlp(c)
                    if c + 1 < NCHUNK:
                        do_gather(c + 1, (2, 3))
                        do_front(c + 1)
                    do_corr(c)
                    do_scatter(c)

    nc.compile()
    return nc


def _get_compiled():
    global _compiled
    if _compiled is None:
        _compiled = _build()
    return _compiled


# ------------------------------ entry point -------------------------------

def _prep_in_maps(coords, propers, encoded, t, answer, W0, b0, W1, b1, W2, b2,
                  W3, b3):
    coords = np.asarray(coords, dtype=np.float32)
    propers_np = np.asarray(propers)
    encoded = np.asarray(encoded, dtype=np.float32)
    t = np.asarray(t, dtype=np.float32)
    W0 = np.asarray(W0, dtype=np.float32)
    b0 = np.asarray(b0, dtype=np.float32)
    W1 = np.asarray(W1, dtype=np.float32)
    b1 = np.asarray(b1, dtype=np.float32)
    W2 = np.asarray(W2, dtype=np.float32)
    b2 = np.asarray(b2, dtype=np.float32)
    W3 = np.asarray(W3, dtype=np.float32)
    b3 = np.asarray(b3, dtype=np.float32)

    eslab = np.zeros((NA, 128), dtype=_BF16)
    eslab[:N_ATOMS] = encoded.astype(_BF16)
    cslab = np.zeros((NA, 64), dtype=np.float32)
    cslab[:N_ATOMS, 0:12] = coords.reshape(N_ATOMS, 12)

    w0all = np.stack([W0[128 * k:128 * (k + 1)] for k in range(4)],
                     axis=1).astype(_BF16)
    # wg4[4t+f, ti, :] selects time ti's geo features only
    wg4 = np.zeros((16, 4, 128), dtype=_BF16)
    for ti in range(T_STEPS):
        wg4[4 * ti + 0, ti] = W0[513].astype(_BF16)   # sin
        wg4[4 * ti + 1, ti] = W0[514].astype(_BF16)   # cos
        wg4[4 * ti + 2, ti] = W0[515].astype(_BF16)   # dl
    bx1 = np.zeros((D, 4), dtype=np.float32)
    for ti in range(T_STEPS):
        bx1[:, ti] = b0 + t[ti] * W0[512]
    bias12 = np.stack([b1, b2], axis=1).astype(np.float32)
    b3h = np.zeros((D, 2), dtype=np.float32)
    b3h[:, 0] = -0.5 * b3[0]
    b3h[:, 1] = 0.5 * b3[1]

    shared = {
        "eslab": eslab,
        "cslab": cslab,
        "w0all": w0all,
        "wg4": wg4,
        "w1": W1.astype(_BF16),
        "w2": W2.astype(_BF16),
        "w3": W3.astype(_BF16),
        "bx1": bx1,
        "bias12": bias12,
        "b3h": b3h,
    }

    props32 = propers_np.astype(np.int32)
    in_maps = []
    for cidx in range(N_CORES):
        shard = np.zeros((PPCT, 4), dtype=np.int32)
        shard[:PPC] = props32[cidx * PPC:(cidx + 1) * PPC]
        order = _order_props(shard, PPC, seed=cidx)
        po = shard[order]
        is_pad = order >= PPC
        gi = np.concatenate([_wrap_idxs(po[:, k]) for k in range(4)], axis=1)
        tgt0 = np.where(is_pad, DUMP, po[:, 0]).astype(np.int32)
        tgt3 = np.where(is_pad, DUMP, po[:, 3]).astype(np.int32)
        si = np.concatenate([_wrap_idxs(tgt0), _wrap_idxs(tgt3)], axis=1)
        in_maps.append({**shared, "gidx": gi, "sidx": si})
    return in_maps


def kernel(coords, propers, encoded, t, answer, W0, b0, W1, b1, W2, b2, W3, b3,
           _trace=False):
    from concourse.bass_utils import run_bass_kernel_spmd

    answer = np.asarray(answer, dtype=np.float32)
    in_maps = _prep_in_maps(coords, propers, encoded, t, answer, W0, b0, W1,
                            b1, W2, b2, W3, b3)
    nc = _get_compiled()
    res = run_bass_kernel_spmd(nc, in_maps, core_ids=list(range(N_CORES)),
                               trace=_trace)
    if _trace:
        kernel.last_exec_ns = res.exec_time_ns
        kernel.last_results = res

    acc = np.zeros((N_ATOMS, 12), dtype=np.float32)
    for cidx in range(N_CORES):
        acc += res.results[cidx]["A0"][:N_ATOMS, :12]
        acc += res.results[cidx]["A3"][:N_ATOMS, :12]
    out = answer + acc.reshape(N_ATOMS, T_STEPS, 3)
    return out.astype(np.float32)


kernel.last_exec_ns = None
kernel.last_results = None


# revision 47
# speedup vs baseline: 1.2538x; 1.0176x over previous
"""Trainium2 Bass kernel for DiffusionPropers (gnn_message_passing), v6.

No device-side table build at all: both gather tables are host inputs.
  - enc^T arrives per chunk via 4 SWDGE transpose-mode gathers straight
    from an HBM [NA, 256B] bf16 table (one 256B descriptor per torsion
    endpoint), giving the [128, props] layout the transposed MLP wants.
  - coords arrive per chunk via 4 non-transposed gathers from an HBM
    [NA, 64] f32 table (256B descriptors) in props layout.
  - Z = sum_k W0_k^T enc_k^T is 8 accumulating matmuls per chunk.
  - all 10 ring transfers per chunk rotate round-robin over the 4 SWDGE
    queues; scatter-add as before.
  - MLP: h1 = identity-broadcast(Z) + masked-wg @ geoT, per-t bias via
    the activation; ti-pair emission so PE/Act overlap; front/mlp/corr
    stage split so DVE geometry of chunk c+1 overlaps MLP of chunk c.
"""
import numpy as np
import ml_dtypes

# ---------------- compile-time constants (hardcoded problem shape) --------
N_ATOMS = 25000
NA = 25088              # padded atoms (196 * 128)
P_TOT = 100000
T_STEPS = 4
D = 128
N_CORES = 8
PPC = 12500             # real props per core
PPCT = 12544            # padded props per core (98 tiles of 128)
NTILES = PPCT // 128    # 98
CH = 896                # props per gather/scatter call (SWDGE ring limit)
NCHUNK = PPCT // CH     # 14
CBLK = CH // 128        # 7
HCOL = CH // 2          # 448 cols per PSUM-bank half
SLAB = 256              # bf16 elems per table slab (512B)
DUMP = NA               # scatter dump row
A_ROWS = NA + 8         # accumulator rows (incl. dump)
A_COLS = 64             # 256B stride for scatter
LEAKY = 0.001

_BF16 = ml_dtypes.bfloat16

_compiled = None


# ------------------------- host-side helpers ------------------------------

def _wrap_idxs(idx: np.ndarray) -> np.ndarray:
    """[n] int -> [128, n/16] int16, wrapped in 16 partitions, replicated x8."""
    n = idx.shape[0]
    assert n % 16 == 0
    w = idx.reshape(-1, 16).T.astype(np.int16)
    return np.tile(w, (8, 1))


def _order_props(props: np.ndarray, n_real: int, seed: int = 0) -> np.ndarray:
    """Order PPCT props so that within every aligned CH-chunk the p0 targets
    are distinct and the p3 targets are distinct (scatter-add race freedom)."""
    n = props.shape[0]
    rng = np.random.default_rng(seed)
    for attempt in range(50):
        perm = rng.permutation(n_real)
        buckets: list[list[int]] = [[] for _ in range(NCHUNK)]
        used0: list[set] = [set() for _ in range(NCHUNK)]
        used3: list[set] = [set() for _ in range(NCHUNK)]
        fail = []
        start = 0
        for j in perm:
            a0 = int(props[j, 0])
            a3 = int(props[j, 3])
            for d in range(NCHUNK):
                b = (start + d) % NCHUNK
                if (len(buckets[b]) < CH and a0 not in used0[b]
                        and a3 not in used3[b]):
                    buckets[b].append(int(j))
                    used0[b].add(a0)
                    used3[b].add(a3)
                    break
            else:
                fail.append(int(j))
            start = (start + 1) % NCHUNK
        if fail:
            continue
        pads = list(range(n_real, n))
        for b in range(NCHUNK):
            while len(buckets[b]) < CH:
                buckets[b].append(pads.pop())
        assert not pads
        order = [j for b in buckets for j in b]
        return np.array(order, dtype=np.int64)
    raise RuntimeError("prop ordering failed")


# ------------------------- device kernel build ----------------------------

def _build():
    import concourse.bass as bass
    import concourse.bacc as bacc
    import concourse.mybir as mybir
    import concourse.tile as tile
    from concourse.masks import make_identity
    from concourse.library_config import mlp as mlp_lib

    F32 = mybir.dt.float32
    BF16 = mybir.dt.bfloat16
    I16 = mybir.dt.int16
    AF = mybir.ActivationFunctionType

    nc = bacc.Bacc("TRN2", target_bir_lowering=False, debug=False,
                   num_devices=N_CORES, num_swdge_queues=4)

    # ---- I/O ----
    eslab = nc.dram_tensor("eslab", [NA, 128], BF16, kind="ExternalInput")
    cslab = nc.dram_tensor("cslab", [NA, 64], F32, kind="ExternalInput")
    w0all = nc.dram_tensor("w0all", [D, 4, 128], BF16, kind="ExternalInput")
    wg4 = nc.dram_tensor("wg4", [16, 4, 128], BF16, kind="ExternalInput")
    w1 = nc.dram_tensor("w1", [D, D], BF16, kind="ExternalInput")
    w2 = nc.dram_tensor("w2", [D, D], BF16, kind="ExternalInput")
    w3 = nc.dram_tensor("w3", [D, 2], BF16, kind="ExternalInput")
    bx1 = nc.dram_tensor("bx1", [D, 4], F32, kind="ExternalInput")
    bias12 = nc.dram_tensor("bias12", [D, 2], F32, kind="ExternalInput")
    b3h = nc.dram_tensor("b3h", [D, 2], F32, kind="ExternalInput")
    gidx = nc.dram_tensor("gidx", [128, 4 * (PPCT // 16)], I16,
                          kind="ExternalInput")
    sidx = nc.dram_tensor("sidx", [128, 2 * (PPCT // 16)], I16,
                          kind="ExternalInput")
    A0 = nc.dram_tensor("A0", [A_ROWS, A_COLS], F32, kind="ExternalOutput")
    A3 = nc.dram_tensor("A3", [A_ROWS, A_COLS], F32, kind="ExternalOutput")
    A0b = nc.dram_tensor("A0b", [A_ROWS, A_COLS], F32, kind="ExternalOutput")
    A3b = nc.dram_tensor("A3b", [A_ROWS, A_COLS], F32, kind="ExternalOutput")

    GI = PPCT // 16     # 784
    GC = CH // 16       # 56

    with tile.TileContext(nc) as tc:
        with (
            tc.tile_pool(name="const", bufs=1) as cpool,
        ):
            nc.gpsimd.load_library(mlp_lib)

            # ---- constants ----
            ibf = cpool.tile([128, 128], BF16)
            make_identity(nc, ibf[:])
            if32 = cpool.tile([128, 128], F32)
            make_identity(nc, if32[:])
            zero_b = cpool.tile([128, 1], F32)
            nc.vector.memset(zero_b[:], 0.0)
            eps_b = cpool.tile([128, 1], F32)
            nc.vector.memset(eps_b[:], 1e-12)
            negh = cpool.tile([128, 1], F32)
            nc.vector.memset(negh[:], -0.5)
            posh = cpool.tile([128, 1], F32)
            nc.vector.memset(posh[:], 0.5)

            w0t = cpool.tile([D, 4, 128], BF16)
            nc.sync.dma_start(out=w0t[:], in_=w0all[:])
            wgt = cpool.tile([16, 4, 128], BF16)
            nc.sync.dma_start(out=wgt[:], in_=wg4[:])
            w1t = cpool.tile([D, D], BF16)
            nc.sync.dma_start(out=w1t[:], in_=w1[:])
            w2t = cpool.tile([D, D], BF16)
            nc.sync.dma_start(out=w2t[:], in_=w2[:])
            w3t = cpool.tile([D, 2], BF16)
            nc.sync.dma_start(out=w3t[:], in_=w3[:])
            bx1t = cpool.tile([D, 4], F32)
            nc.sync.dma_start(out=bx1t[:], in_=bx1[:])
            b12t = cpool.tile([D, 2], F32)
            nc.sync.dma_start(out=b12t[:], in_=bias12[:])
            b3t = cpool.tile([D, 2], F32)
            nc.sync.dma_start(out=b3t[:], in_=b3h[:])
            gixt = cpool.tile([128, 4 * GI], I16)
            nc.sync.dma_start(out=gixt[:], in_=gidx[:])
            sixt = cpool.tile([128, 2 * GI], I16)
            nc.sync.dma_start(out=sixt[:], in_=sidx[:])

            # ================= Phase 1: main loop =================
            with (
                tc.tile_pool(name="gat", bufs=3) as gpool,
                tc.tile_pool(name="mlp", bufs=2) as mpool,
                tc.tile_pool(name="geo", bufs=2) as geopool,
                tc.tile_pool(name="cto", bufs=3) as ctpool,
                tc.tile_pool(name="hps", bufs=2, space="PSUM") as hps,
                tc.tile_pool(name="msc", bufs=2, space="PSUM") as mscps,
                tc.tile_pool(name="dt", bufs=1, space="PSUM") as dtps,
            ):
                Gof = {}
                ctof = {}
                qctr = [0]

                def next_q():
                    q = qctr[0] % 4
                    qctr[0] += 1
                    return q

                def do_gather(c, ks):
                    E, C = Gof.setdefault(c, ([], []))
                    for k in ks:
                        ge = gpool.tile([128, 1, CH], BF16, tag=f"ge{k}")
                        nc.gpsimd.dma_gather(
                            ge[:], eslab[:],
                            gixt[:, k * GI + c * GC:
                                 k * GI + (c + 1) * GC],
                            CH, CH, 128, transpose=True,
                            queue_num=next_q())
                        E.append(ge)
                        gc = gpool.tile([128, CBLK, 64], F32, tag=f"gc{k}")
                        nc.gpsimd.dma_gather(
                            gc[:], cslab[:],
                            gixt[:, k * GI + c * GC:
                                 k * GI + (c + 1) * GC],
                            CH, CH, 64, queue_num=next_q())
                        C.append(gc)

                front_of = {}

                def do_front(c):
                    E, C = Gof[c]
                    cco = [C[k][:, :, 0:12] for k in range(4)]

                    # ---- Z^T = sum_k W0_k^T enc_k^T (8 matmuls) ----
                    zbf = mpool.tile([128, CH], BF16, tag="zbf")
                    for h in range(2):
                        sl = slice(h * HCOL, (h + 1) * HCOL)
                        zps = mscps.tile([128, 512], F32, tag="msc")
                        for k in range(4):
                            nc.tensor.matmul(
                                zps[:, 0:HCOL],
                                lhsT=w0t[:, k, :], rhs=E[k][:, 0, sl],
                                start=(k == 0), stop=(k == 3))
                        nc.vector.tensor_copy(zbf[:, sl], zps[:, 0:HCOL])

                    # ---- geometry (props layout) ----
                    u1 = geopool.tile([128, CBLK, 12], F32, tag="u1")
                    u2 = geopool.tile([128, CBLK, 12], F32, tag="u2")
                    u3 = geopool.tile([128, CBLK, 12], F32, tag="u3")
                    dr = geopool.tile([128, CBLK, 12], F32, tag="dr")
                    nc.vector.tensor_sub(u1[:], cco[1], cco[0])
                    nc.vector.tensor_sub(u2[:], cco[2], cco[1])
                    nc.vector.tensor_sub(u3[:], cco[3], cco[2])
                    nc.vector.tensor_sub(dr[:], cco[0], cco[3])

                    def cross(out, a, b):
                        tmp = geopool.tile([128, CBLK, 4], F32, tag="ctmp")
                        for x in range(3):
                            y, z = (x + 1) % 3, (x + 2) % 3
                            nc.vector.tensor_mul(tmp[:], a[:, :, y::3],
                                                 b[:, :, z::3])
                            nc.vector.tensor_mul(out[:, :, x::3],
                                                 a[:, :, z::3], b[:, :, y::3])
                            nc.vector.tensor_sub(out[:, :, x::3], tmp[:],
                                                 out[:, :, x::3])

                    cr12 = geopool.tile([128, CBLK, 12], F32, tag="cr12")
                    cr23 = geopool.tile([128, CBLK, 12], F32, tag="cr23")
                    cross(cr12, u1, u2)
                    cross(cr23, u2, u3)

                    def dot3(out, a, b, tmp):
                        nc.vector.tensor_mul(tmp[:], a[:], b[:])
                        nc.vector.tensor_add(out[:], tmp[:, :, 0::3],
                                             tmp[:, :, 1::3])
                        nc.vector.tensor_add(out[:], out[:], tmp[:, :, 2::3])

                    tmp12 = geopool.tile([128, CBLK, 12], F32, tag="tmp12")
                    n2 = geopool.tile([128, CBLK, 4], F32, tag="n2")
                    dot3(n2, u2, u2, tmp12)
                    nc.scalar.activation(n2[:], n2[:], AF.Sqrt, bias=zero_b[:])
                    sn = geopool.tile([128, CBLK, 4], F32, tag="sn")
                    dot3(sn, u1, cr23, tmp12)
                    nc.vector.tensor_mul(sn[:], sn[:], n2[:])
                    cn = geopool.tile([128, CBLK, 4], F32, tag="cn")
                    dot3(cn, cr12, cr23, tmp12)
                    hy = geopool.tile([128, CBLK, 4], F32, tag="hy")
                    t2 = geopool.tile([128, CBLK, 4], F32, tag="t2")
                    nc.vector.tensor_mul(hy[:], sn[:], sn[:])
                    nc.vector.tensor_mul(t2[:], cn[:], cn[:])
                    nc.vector.tensor_add(hy[:], hy[:], t2[:])
                    nc.scalar.activation(hy[:], hy[:], AF.Sqrt, bias=eps_b[:])
                    rh = geopool.tile([128, CBLK, 4], F32, tag="rh")
                    nc.vector.reciprocal(rh[:], hy[:])
                    dl = geopool.tile([128, CBLK, 4], F32, tag="dl")
                    dot3(dl, dr, dr, tmp12)
                    nc.scalar.activation(dl[:], dl[:], AF.Sqrt, bias=eps_b[:])
                    rdl = geopool.tile([128, CBLK, 4], F32, tag="rdl")
                    nc.vector.reciprocal(rdl[:], dl[:])
                    dh = geopool.tile([128, CBLK, 12], F32, tag="dh")
                    for x in range(3):
                        nc.vector.tensor_mul(dh[:, :, x::3], dr[:, :, x::3],
                                             rdl[:])
                    # geo features per (prop, t): [sin, cos, dl, pad]
                    geo = geopool.tile([128, CBLK, 16], F32, tag="geo")
                    nc.vector.memset(geo[:, :, 3::4], 0.0)
                    nc.vector.tensor_mul(geo[:, :, 0::4], sn[:], rh[:])
                    nc.vector.tensor_mul(geo[:, :, 1::4], cn[:], rh[:])
                    nc.vector.tensor_copy(geo[:, :, 2::4], dl[:])

                    # geoT [16, 896]
                    geoT = mpool.tile([16, CH], BF16, tag="geoT")
                    for hb, nb in ((0, 4), (4, 3)):
                        gtp = mscps.tile([128, 512], F32, tag="msc")
                        for b in range(nb):
                            nc.tensor.matmul(
                                gtp[0:16, b * 128:(b + 1) * 128],
                                lhsT=geo[:, hb + b, :],
                                rhs=if32[:], is_transpose=True,
                                start=True, stop=True)
                        nc.vector.tensor_copy(
                            geoT[:, hb * 128:(hb + nb) * 128],
                            gtp[0:16, 0:nb * 128])

                    front_of[c] = (zbf, geoT, dh)

                def do_mlp(c):
                    zbf, geoT, dh = front_of[c]

                    # ---- per-t MLP (ti pairs, layer-major in pair) ----
                    dtc = dtps.tile([128, CBLK, 4, 2], F32, tag="dtc")

                    def mm_h1(ti):
                        h1 = hps.tile([128, 2, 512], F32, tag="h")
                        for h in range(2):
                            sl = slice(h * HCOL, (h + 1) * HCOL)
                            nc.tensor.matmul(
                                h1[:, h, 0:HCOL], lhsT=ibf[:],
                                rhs=zbf[:, sl], start=True, stop=False)
                            nc.tensor.matmul(
                                h1[:, h, 0:HCOL],
                                lhsT=wgt[:, ti, :],
                                rhs=geoT[:, sl],
                                start=False, stop=True)
                        return h1

                    def mm_layer(w, x):
                        hp = hps.tile([128, 2, 512], F32, tag="h")
                        for h in range(2):
                            nc.tensor.matmul(hp[:, h, 0:HCOL], lhsT=w[:],
                                             rhs=x[:, h, :],
                                             start=True, stop=True)
                        return hp

                    def act(hp, bias, tag):
                        x = mpool.tile([128, 2, HCOL], BF16, tag=tag)
                        nc.scalar.activation(x[:], hp[:, :, 0:HCOL],
                                             AF.Prelu, bias=bias, alpha=LEAKY)
                        return x

                    for t0 in (0, 2):
                        h1a = mm_h1(t0)
                        h1b = mm_h1(t0 + 1)
                        x1a = act(h1a, bx1t[:, t0:t0 + 1], "x1a")
                        x1b = act(h1b, bx1t[:, t0 + 1:t0 + 2], "x1b")
                        h2a = mm_layer(w1t, x1a)
                        h2b = mm_layer(w1t, x1b)
                        x2a = act(h2a, b12t[:, 0:1], "x2a")
                        x2b = act(h2b, b12t[:, 0:1], "x2b")
                        h3a = mm_layer(w2t, x2a)
                        h3b = mm_layer(w2t, x2b)
                        for ti, h3 in ((t0, h3a), (t0 + 1, h3b)):
                            x3 = mpool.tile([128, CH], BF16,
                                            tag=f"x3{ti % 2}")
                            nc.scalar.activation(
                                x3[:].rearrange("p (h c) -> p h c", h=2),
                                h3[:, :, 0:HCOL], AF.Prelu,
                                bias=b12t[:, 1:2], alpha=LEAKY)
                            for b in range(CBLK):
                                nc.tensor.matmul(
                                    dtc[:, b, ti, :],
                                    lhsT=x3[:, b * 128:(b + 1) * 128],
                                    rhs=w3t[:], start=True, stop=True)
                    front_of[c] = (dtc, dh)

                def do_corr(c):
                    dtc, dh = front_of.pop(c)
                    c0t = ctpool.tile([128, CBLK, 12], F32, tag="c0t")
                    c3t = ctpool.tile([128, CBLK, 12], F32, tag="c3t")
                    s0 = geopool.tile([128, CBLK, 4], F32, tag="s0")
                    s3 = geopool.tile([128, CBLK, 4], F32, tag="s3")
                    nc.vector.tensor_scalar(
                        s0[:], dtc[:, :, :, 0], scalar1=negh[:],
                        scalar2=b3t[:, 0:1],
                        op0=mybir.AluOpType.mult, op1=mybir.AluOpType.add)
                    nc.vector.tensor_scalar(
                        s3[:], dtc[:, :, :, 1], scalar1=posh[:],
                        scalar2=b3t[:, 1:2],
                        op0=mybir.AluOpType.mult, op1=mybir.AluOpType.add)
                    for x in range(3):
                        nc.vector.tensor_mul(c0t[:, :, x::3], dh[:, :, x::3],
                                             s0[:])
                        nc.vector.tensor_mul(c3t[:, :, x::3], dh[:, :, x::3],
                                             s3[:])
                    ctof[c] = (c0t, c3t)

                def do_scatter(c):
                    c0t, c3t = ctof.pop(c)
                    # alternate accumulators so consecutive scatter-adds to
                    # the same HBM tensor don't chain on WAW DMA completion
                    d0 = A0 if c % 2 == 0 else A0b
                    d3 = A3 if c % 2 == 0 else A3b
                    nc.gpsimd.dma_scatter_add(
                        d0[:, :12], c0t[:],
                        sixt[:, c * GC:(c + 1) * GC],
                        CH, CH, 12, elem_step=A_COLS,
                        queue_num=next_q())
                    nc.gpsimd.dma_scatter_add(
                        d3[:, :12], c3t[:],
                        sixt[:, GI + c * GC:GI + (c + 1) * GC],
                        CH, CH, 12, elem_step=A_COLS,
                        queue_num=next_q())
                    del Gof[c]

                # pipeline: gathers 1 ahead; front(c) overlaps mlp(c-1) on
                # DVE/PE; corrections+scatter of c-1 emitted before mlp(c)
                do_gather(0, (0, 1, 2, 3))
                do_front(0)
                for c in range(NCHUNK):
                    if c + 1 < NCHUNK:
                        do_gather(c + 1, (0, 1))
                    do_mlp(c)
                    if c + 1 < NCHUNK:
                        do_gather(c + 1, (2, 3))
                        do_front(c + 1)
                    do_corr(c)
                    if c >= 1:
                        do_scatter(c - 1)
                do_scatter(NCHUNK - 1)

    nc.compile()
    return nc


def _get_compiled():
    global _compiled
    if _compiled is None:
        _compiled = _build()
    return _compiled


# ------------------------------ entry point -------------------------------

def _prep_in_maps(coords, propers, encoded, t, answer, W0, b0, W1, b1, W2, b2,
                  W3, b3):
    coords = np.asarray(coords, dtype=np.float32)
    propers_np = np.asarray(propers)
    encoded = np.asarray(encoded, dtype=np.float32)
    t = np.asarray(t, dtype=np.float32)
    W0 = np.asarray(W0, dtype=np.float32)
    b0 = np.asarray(b0, dtype=np.float32)
    W1 = np.asarray(W1, dtype=np.float32)
    b1 = np.asarray(b1, dtype=np.float32)
    W2 = np.asarray(W2, dtype=np.float32)
    b2 = np.asarray(b2, dtype=np.float32)
    W3 = np.asarray(W3, dtype=np.float32)
    b3 = np.asarray(b3, dtype=np.float32)

    eslab = np.zeros((NA, 128), dtype=_BF16)
    eslab[:N_ATOMS] = encoded.astype(_BF16)
    cslab = np.zeros((NA, 64), dtype=np.float32)
    cslab[:N_ATOMS, 0:12] = coords.reshape(N_ATOMS, 12)

    w0all = np.stack([W0[128 * k:128 * (k + 1)] for k in range(4)],
                     axis=1).astype(_BF16)
    # wg4[4t+f, ti, :] selects time ti's geo features only
    wg4 = np.zeros((16, 4, 128), dtype=_BF16)
    for ti in range(T_STEPS):
        wg4[4 * ti + 0, ti] = W0[513].astype(_BF16)   # sin
        wg4[4 * ti + 1, ti] = W0[514].astype(_BF16)   # cos
        wg4[4 * ti + 2, ti] = W0[515].astype(_BF16)   # dl
    bx1 = np.zeros((D, 4), dtype=np.float32)
    for ti in range(T_STEPS):
        bx1[:, ti] = b0 + t[ti] * W0[512]
    bias12 = np.stack([b1, b2], axis=1).astype(np.float32)
    b3h = np.zeros((D, 2), dtype=np.float32)
    b3h[:, 0] = -0.5 * b3[0]
    b3h[:, 1] = 0.5 * b3[1]

    shared = {
        "eslab": eslab,
        "cslab": cslab,
        "w0all": w0all,
        "wg4": wg4,
        "w1": W1.astype(_BF16),
        "w2": W2.astype(_BF16),
        "w3": W3.astype(_BF16),
        "bx1": bx1,
        "bias12": bias12,
        "b3h": b3h,
    }

    props32 = propers_np.astype(np.int32)
    in_maps = []
    for cidx in range(N_CORES):
        shard = np.zeros((PPCT, 4), dtype=np.int32)
        shard[:PPC] = props32[cidx * PPC:(cidx + 1) * PPC]
        order = _order_props(shard, PPC, seed=cidx)
        po = shard[order]
        is_pad = order >= PPC
        gi = np.concatenate([_wrap_idxs(po[:, k]) for k in range(4)], axis=1)
        tgt0 = np.where(is_pad, DUMP, po[:, 0]).astype(np.int32)
        tgt3 = np.where(is_pad, DUMP, po[:, 3]).astype(np.int32)
        si = np.concatenate([_wrap_idxs(tgt0), _wrap_idxs(tgt3)], axis=1)
        in_maps.append({**shared, "gidx": gi, "sidx": si})
    return in_maps


def kernel(coords, propers, encoded, t, answer, W0, b0, W1, b1, W2, b2, W3, b3,
           _trace=False):
    from concourse.bass_utils import run_bass_kernel_spmd

    answer = np.asarray(answer, dtype=np.float32)
    in_maps = _prep_in_maps(coords, propers, encoded, t, answer, W0, b0, W1,
                            b1, W2, b2, W3, b3)
    nc = _get_compiled()
    res = run_bass_kernel_spmd(nc, in_maps, core_ids=list(range(N_CORES)),
                               trace=_trace)
    if _trace:
        kernel.last_exec_ns = res.exec_time_ns
        kernel.last_results = res

    acc = np.zeros((N_ATOMS, 12), dtype=np.float32)
    for cidx in range(N_CORES):
        for name in ("A0", "A3", "A0b", "A3b"):
            acc += res.results[cidx][name][:N_ATOMS, :12]
    out = answer + acc.reshape(N_ATOMS, T_STEPS, 3)
    return out.astype(np.float32)


kernel.last_exec_ns = None
kernel.last_results = None


# revision 48
# speedup vs baseline: 1.2571x; 1.0026x over previous
"""Trainium2 Bass kernel for DiffusionPropers (gnn_message_passing), v6.

No device-side table build at all: both gather tables are host inputs.
  - enc^T arrives per chunk via 4 SWDGE transpose-mode gathers straight
    from an HBM [NA, 256B] bf16 table (one 256B descriptor per torsion
    endpoint), giving the [128, props] layout the transposed MLP wants.
  - coords arrive per chunk via 4 non-transposed gathers from an HBM
    [NA, 64] f32 table (256B descriptors) in props layout.
  - Z = sum_k W0_k^T enc_k^T is 8 accumulating matmuls per chunk.
  - all 10 ring transfers per chunk rotate round-robin over the 4 SWDGE
    queues; scatter-add as before.
  - MLP: h1 = identity-broadcast(Z) + masked-wg @ geoT, per-t bias via
    the activation; ti-pair emission so PE/Act overlap; front/mlp/corr
    stage split so DVE geometry of chunk c+1 overlaps MLP of chunk c.
"""
import numpy as np
import ml_dtypes

# ---------------- compile-time constants (hardcoded problem shape) --------
N_ATOMS = 25000
NA = 25088              # padded atoms (196 * 128)
P_TOT = 100000
T_STEPS = 4
D = 128
N_CORES = 8
PPC = 12500             # real props per core
PPCT = 12544            # padded props per core (98 tiles of 128)
NTILES = PPCT // 128    # 98
CH = 896                # props per gather/scatter call (SWDGE ring limit)
NCHUNK = PPCT // CH     # 14
CBLK = CH // 128        # 7
HCOL = CH // 2          # 448 cols per PSUM-bank half
SLAB = 256              # bf16 elems per table slab (512B)
DUMP = NA               # scatter dump row
A_ROWS = NA + 8         # accumulator rows (incl. dump)
A_COLS = 64             # 256B stride for scatter
LEAKY = 0.001

_BF16 = ml_dtypes.bfloat16

_compiled = None


# ------------------------- host-side helpers ------------------------------

def _wrap_idxs(idx: np.ndarray) -> np.ndarray:
    """[n] int -> [128, n/16] int16, wrapped in 16 partitions, replicated x8."""
    n = idx.shape[0]
    assert n % 16 == 0
    w = idx.reshape(-1, 16).T.astype(np.int16)
    return np.tile(w, (8, 1))


def _order_props(props: np.ndarray, n_real: int, seed: int = 0) -> np.ndarray:
    """Order PPCT props so that within every aligned CH-chunk the p0 targets
    are distinct and the p3 targets are distinct (scatter-add race freedom)."""
    n = props.shape[0]
    rng = np.random.default_rng(seed)
    for attempt in range(50):
        perm = rng.permutation(n_real)
        buckets: list[list[int]] = [[] for _ in range(NCHUNK)]
        used0: list[set] = [set() for _ in range(NCHUNK)]
        used3: list[set] = [set() for _ in range(NCHUNK)]
        fail = []
        start = 0
        for j in perm:
            a0 = int(props[j, 0])
            a3 = int(props[j, 3])
            for d in range(NCHUNK):
                b = (start + d) % NCHUNK
                if (len(buckets[b]) < CH and a0 not in used0[b]
                        and a3 not in used3[b]):
                    buckets[b].append(int(j))
                    used0[b].add(a0)
                    used3[b].add(a3)
                    break
            else:
                fail.append(int(j))
            start = (start + 1) % NCHUNK
        if fail:
            continue
        pads = list(range(n_real, n))
        for b in range(NCHUNK):
            while len(buckets[b]) < CH:
                buckets[b].append(pads.pop())
        assert not pads
        order = [j for b in buckets for j in b]
        return np.array(order, dtype=np.int64)
    raise RuntimeError("prop ordering failed")


# ------------------------- device kernel build ----------------------------

def _build():
    import concourse.bass as bass
    import concourse.bacc as bacc
    import concourse.mybir as mybir
    import concourse.tile as tile
    from concourse.masks import make_identity
    from concourse.library_config import mlp as mlp_lib

    F32 = mybir.dt.float32
    BF16 = mybir.dt.bfloat16
    I16 = mybir.dt.int16
    AF = mybir.ActivationFunctionType

    nc = bacc.Bacc("TRN2", target_bir_lowering=False, debug=False,
                   num_devices=N_CORES, num_swdge_queues=4)

    # ---- I/O ----
    eslab = nc.dram_tensor("eslab", [NA, 128], BF16, kind="ExternalInput")
    cslab = nc.dram_tensor("cslab", [NA, 64], F32, kind="ExternalInput")
    w0all = nc.dram_tensor("w0all", [D, 4, 128], BF16, kind="ExternalInput")
    wg4 = nc.dram_tensor("wg4", [16, 4, 128], BF16, kind="ExternalInput")
    w1 = nc.dram_tensor("w1", [D, D], BF16, kind="ExternalInput")
    w2 = nc.dram_tensor("w2", [D, D], BF16, kind="ExternalInput")
    w3 = nc.dram_tensor("w3", [D, 2], BF16, kind="ExternalInput")
    bx1 = nc.dram_tensor("bx1", [D, 4], F32, kind="ExternalInput")
    bias12 = nc.dram_tensor("bias12", [D, 2], F32, kind="ExternalInput")
    b3h = nc.dram_tensor("b3h", [D, 2], F32, kind="ExternalInput")
    gidx = nc.dram_tensor("gidx", [128, 4 * (PPCT // 16)], I16,
                          kind="ExternalInput")
    sidx = nc.dram_tensor("sidx", [128, 2 * (PPCT // 16)], I16,
                          kind="ExternalInput")
    A0 = nc.dram_tensor("A0", [A_ROWS, A_COLS], F32, kind="ExternalOutput")
    A3 = nc.dram_tensor("A3", [A_ROWS, A_COLS], F32, kind="ExternalOutput")
    A0b = nc.dram_tensor("A0b", [A_ROWS, A_COLS], F32, kind="ExternalOutput")
    A3b = nc.dram_tensor("A3b", [A_ROWS, A_COLS], F32, kind="ExternalOutput")

    GI = PPCT // 16     # 784
    GC = CH // 16       # 56

    with tile.TileContext(nc) as tc:
        with (
            tc.tile_pool(name="const", bufs=1) as cpool,
        ):
            nc.gpsimd.load_library(mlp_lib)

            # ---- constants ----
            ibf = cpool.tile([128, 128], BF16)
            make_identity(nc, ibf[:])
            if32 = cpool.tile([128, 128], F32)
            make_identity(nc, if32[:])
            zero_b = cpool.tile([128, 1], F32)
            nc.vector.memset(zero_b[:], 0.0)
            eps_b = cpool.tile([128, 1], F32)
            nc.vector.memset(eps_b[:], 1e-12)
            negh = cpool.tile([128, 1], F32)
            nc.vector.memset(negh[:], -0.5)
            posh = cpool.tile([128, 1], F32)
            nc.vector.memset(posh[:], 0.5)

            w0t = cpool.tile([D, 4, 128], BF16)
            nc.sync.dma_start(out=w0t[:], in_=w0all[:])
            wgt = cpool.tile([16, 4, 128], BF16)
            nc.sync.dma_start(out=wgt[:], in_=wg4[:])
            w1t = cpool.tile([D, D], BF16)
            nc.sync.dma_start(out=w1t[:], in_=w1[:])
            w2t = cpool.tile([D, D], BF16)
            nc.sync.dma_start(out=w2t[:], in_=w2[:])
            w3t = cpool.tile([D, 2], BF16)
            nc.sync.dma_start(out=w3t[:], in_=w3[:])
            bx1t = cpool.tile([D, 4], F32)
            nc.sync.dma_start(out=bx1t[:], in_=bx1[:])
            b12t = cpool.tile([D, 2], F32)
            nc.sync.dma_start(out=b12t[:], in_=bias12[:])
            b3t = cpool.tile([D, 2], F32)
            nc.sync.dma_start(out=b3t[:], in_=b3h[:])
            gixt = cpool.tile([128, 4 * GI], I16)
            nc.sync.dma_start(out=gixt[:], in_=gidx[:])
            sixt = cpool.tile([128, 2 * GI], I16)
            nc.sync.dma_start(out=sixt[:], in_=sidx[:])

            # ================= Phase 1: main loop =================
            with (
                tc.tile_pool(name="gat", bufs=3) as gpool,
                tc.tile_pool(name="mlp", bufs=2) as mpool,
                tc.tile_pool(name="geo", bufs=2) as geopool,
                tc.tile_pool(name="cto", bufs=3) as ctpool,
                tc.tile_pool(name="hps", bufs=2, space="PSUM") as hps,
                tc.tile_pool(name="msc", bufs=2, space="PSUM") as mscps,
                tc.tile_pool(name="dt", bufs=1, space="PSUM") as dtps,
            ):
                Gof = {}
                ctof = {}
                qctr = [0]

                def next_q():
                    q = qctr[0] % 4
                    qctr[0] += 1
                    return q

                def do_gather(c, ks):
                    E, C = Gof.setdefault(c, ([], []))
                    for k in ks:
                        ge = gpool.tile([128, 1, CH], BF16, tag=f"ge{k}")
                        nc.gpsimd.dma_gather(
                            ge[:], eslab[:],
                            gixt[:, k * GI + c * GC:
                                 k * GI + (c + 1) * GC],
                            CH, CH, 128, transpose=True,
                            queue_num=next_q())
                        E.append(ge)
                        gc = gpool.tile([128, CBLK, 64], F32, tag=f"gc{k}")
                        nc.gpsimd.dma_gather(
                            gc[:], cslab[:],
                            gixt[:, k * GI + c * GC:
                                 k * GI + (c + 1) * GC],
                            CH, CH, 64, queue_num=next_q())
                        C.append(gc)

                front_of = {}

                def do_front(c):
                    E, C = Gof[c]
                    cco = [C[k][:, :, 0:12] for k in range(4)]

                    # ---- Z^T = sum_k W0_k^T enc_k^T (8 matmuls) ----
                    zbf = mpool.tile([128, CH], BF16, tag="zbf")
                    for h in range(2):
                        sl = slice(h * HCOL, (h + 1) * HCOL)
                        zps = mscps.tile([128, 512], F32, tag="msc")
                        for k in range(4):
                            nc.tensor.matmul(
                                zps[:, 0:HCOL],
                                lhsT=w0t[:, k, :], rhs=E[k][:, 0, sl],
                                start=(k == 0), stop=(k == 3))
                        nc.vector.tensor_copy(zbf[:, sl], zps[:, 0:HCOL])

                    # ---- geometry (props layout) ----
                    u1 = geopool.tile([128, CBLK, 12], F32, tag="u1")
                    u2 = geopool.tile([128, CBLK, 12], F32, tag="u2")
                    u3 = geopool.tile([128, CBLK, 12], F32, tag="u3")
                    dr = geopool.tile([128, CBLK, 12], F32, tag="dr")
                    nc.vector.tensor_sub(u1[:], cco[1], cco[0])
                    nc.vector.tensor_sub(u2[:], cco[2], cco[1])
                    nc.vector.tensor_sub(u3[:], cco[3], cco[2])
                    nc.vector.tensor_sub(dr[:], cco[0], cco[3])

                    def cross(out, a, b):
                        tmp = geopool.tile([128, CBLK, 4], F32, tag="ctmp")
                        for x in range(3):
                            y, z = (x + 1) % 3, (x + 2) % 3
                            nc.vector.tensor_mul(tmp[:], a[:, :, y::3],
                                                 b[:, :, z::3])
                            nc.vector.tensor_mul(out[:, :, x::3],
                                                 a[:, :, z::3], b[:, :, y::3])
                            nc.vector.tensor_sub(out[:, :, x::3], tmp[:],
                                                 out[:, :, x::3])

                    cr12 = geopool.tile([128, CBLK, 12], F32, tag="cr12")
                    cr23 = geopool.tile([128, CBLK, 12], F32, tag="cr23")
                    cross(cr12, u1, u2)
                    cross(cr23, u2, u3)

                    def dot3(out, a, b, tmp):
                        nc.vector.tensor_mul(tmp[:], a[:], b[:])
                        nc.vector.tensor_add(out[:], tmp[:, :, 0::3],
                                             tmp[:, :, 1::3])
                        nc.vector.tensor_add(out[:], out[:], tmp[:, :, 2::3])

                    tmp12 = geopool.tile([128, CBLK, 12], F32, tag="tmp12")
                    n2 = geopool.tile([128, CBLK, 4], F32, tag="n2")
                    dot3(n2, u2, u2, tmp12)
                    nc.scalar.activation(n2[:], n2[:], AF.Sqrt, bias=zero_b[:])
                    sn = geopool.tile([128, CBLK, 4], F32, tag="sn")
                    dot3(sn, u1, cr23, tmp12)
                    nc.vector.tensor_mul(sn[:], sn[:], n2[:])
                    cn = geopool.tile([128, CBLK, 4], F32, tag="cn")
                    dot3(cn, cr12, cr23, tmp12)
                    hy = geopool.tile([128, CBLK, 4], F32, tag="hy")
                    t2 = geopool.tile([128, CBLK, 4], F32, tag="t2")
                    nc.vector.tensor_mul(hy[:], sn[:], sn[:])
                    nc.vector.tensor_mul(t2[:], cn[:], cn[:])
                    nc.vector.tensor_add(hy[:], hy[:], t2[:])
                    nc.scalar.activation(hy[:], hy[:], AF.Sqrt, bias=eps_b[:])
                    rh = geopool.tile([128, CBLK, 4], F32, tag="rh")
                    nc.vector.reciprocal(rh[:], hy[:])
                    dl = geopool.tile([128, CBLK, 4], F32, tag="dl")
                    dot3(dl, dr, dr, tmp12)
                    nc.scalar.activation(dl[:], dl[:], AF.Sqrt, bias=eps_b[:])
                    rdl = geopool.tile([128, CBLK, 4], F32, tag="rdl")
                    nc.vector.reciprocal(rdl[:], dl[:])
                    dh = geopool.tile([128, CBLK, 12], F32, tag="dh")
                    for x in range(3):
                        nc.vector.tensor_mul(dh[:, :, x::3], dr[:, :, x::3],
                                             rdl[:])
                    # geo features per (prop, t): [sin, cos, dl, pad]
                    geo = geopool.tile([128, CBLK, 16], F32, tag="geo")
                    nc.vector.memset(geo[:, :, 3::4], 0.0)
                    nc.vector.tensor_mul(geo[:, :, 0::4], sn[:], rh[:])
                    nc.vector.tensor_mul(geo[:, :, 1::4], cn[:], rh[:])
                    nc.vector.tensor_copy(geo[:, :, 2::4], dl[:])

                    # geoT [16, 896]
                    geoT = mpool.tile([16, CH], BF16, tag="geoT")
                    for hb, nb in ((0, 4), (4, 3)):
                        gtp = mscps.tile([128, 512], F32, tag="msc")
                        for b in range(nb):
                            nc.tensor.matmul(
                                gtp[0:16, b * 128:(b + 1) * 128],
                                lhsT=geo[:, hb + b, :],
                                rhs=if32[:], is_transpose=True,
                                start=True, stop=True)
                        nc.vector.tensor_copy(
                            geoT[:, hb * 128:(hb + nb) * 128],
                            gtp[0:16, 0:nb * 128])

                    front_of[c] = (zbf, geoT, dh)

                def do_mlp(c):
                    zbf, geoT, dh = front_of[c]

                    # ---- per-t MLP (ti pairs, layer-major in pair) ----
                    dtc = dtps.tile([128, CBLK, 4, 2], F32, tag="dtc")

                    def mm_h1(ti):
                        h1 = hps.tile([128, 2, 512], F32, tag="h")
                        for h in range(2):
                            sl = slice(h * HCOL, (h + 1) * HCOL)
                            nc.tensor.matmul(
                                h1[:, h, 0:HCOL], lhsT=ibf[:],
                                rhs=zbf[:, sl], start=True, stop=False)
                            nc.tensor.matmul(
                                h1[:, h, 0:HCOL],
                                lhsT=wgt[:, ti, :],
                                rhs=geoT[:, sl],
                                start=False, stop=True)
                        return h1

                    def mm_layer(w, x):
                        hp = hps.tile([128, 2, 512], F32, tag="h")
                        for h in range(2):
                            nc.tensor.matmul(hp[:, h, 0:HCOL], lhsT=w[:],
                                             rhs=x[:, h, :],
                                             start=True, stop=True)
                        return hp

                    def act(hp, bias, tag):
                        x = mpool.tile([128, 2, HCOL], BF16, tag=tag)
                        nc.scalar.activation(x[:], hp[:, :, 0:HCOL],
                                             AF.Prelu, bias=bias, alpha=LEAKY)
                        return x

                    for t0 in (0, 2):
                        h1a = mm_h1(t0)
                        h1b = mm_h1(t0 + 1)
                        x1a = act(h1a, bx1t[:, t0:t0 + 1], "x1a")
                        x1b = act(h1b, bx1t[:, t0 + 1:t0 + 2], "x1b")
                        h2a = mm_layer(w1t, x1a)
                        h2b = mm_layer(w1t, x1b)
                        x2a = act(h2a, b12t[:, 0:1], "x2a")
                        x2b = act(h2b, b12t[:, 0:1], "x2b")
                        h3a = mm_layer(w2t, x2a)
                        h3b = mm_layer(w2t, x2b)
                        for ti, h3 in ((t0, h3a), (t0 + 1, h3b)):
                            x3 = mpool.tile([128, CH], BF16,
                                            tag=f"x3{ti % 2}")
                            nc.scalar.activation(
                                x3[:].rearrange("p (h c) -> p h c", h=2),
                                h3[:, :, 0:HCOL], AF.Prelu,
                                bias=b12t[:, 1:2], alpha=LEAKY)
                            for b in range(CBLK):
                                nc.tensor.matmul(
                                    dtc[:, b, ti, :],
                                    lhsT=x3[:, b * 128:(b + 1) * 128],
                                    rhs=w3t[:], start=True, stop=True)
                    front_of[c] = (dtc, dh)

                def do_corr(c):
                    dtc, dh = front_of.pop(c)
                    c0t = ctpool.tile([128, CBLK, 12], F32, tag="c0t")
                    c3t = ctpool.tile([128, CBLK, 12], F32, tag="c3t")
                    s0 = geopool.tile([128, CBLK, 4], F32, tag="s0")
                    s3 = geopool.tile([128, CBLK, 4], F32, tag="s3")
                    nc.vector.tensor_scalar(
                        s0[:], dtc[:, :, :, 0], scalar1=negh[:],
                        scalar2=b3t[:, 0:1],
                        op0=mybir.AluOpType.mult, op1=mybir.AluOpType.add)
                    nc.vector.tensor_scalar(
                        s3[:], dtc[:, :, :, 1], scalar1=posh[:],
                        scalar2=b3t[:, 1:2],
                        op0=mybir.AluOpType.mult, op1=mybir.AluOpType.add)
                    for x in range(3):
                        nc.vector.tensor_mul(c0t[:, :, x::3], dh[:, :, x::3],
                                             s0[:])
                        nc.vector.tensor_mul(c3t[:, :, x::3], dh[:, :, x::3],
                                             s3[:])
                    ctof[c] = (c0t, c3t)

                def do_scatter(c):
                    c0t, c3t = ctof.pop(c)
                    # alternate accumulators so consecutive scatter-adds to
                    # the same HBM tensor don't chain on WAW DMA completion
                    d0 = A0 if c % 2 == 0 else A0b
                    d3 = A3 if c % 2 == 0 else A3b
                    nc.gpsimd.dma_scatter_add(
                        d0[:, :12], c0t[:],
                        sixt[:, c * GC:(c + 1) * GC],
                        CH, CH, 12, elem_step=A_COLS,
                        queue_num=next_q())
                    nc.gpsimd.dma_scatter_add(
                        d3[:, :12], c3t[:],
                        sixt[:, GI + c * GC:GI + (c + 1) * GC],
                        CH, CH, 12, elem_step=A_COLS,
                        queue_num=next_q())
                    del Gof[c]

                # pipeline: gathers 1 ahead; front(c) overlaps mlp(c-1) on
                # DVE/PE; corrections+scatter of c-1 emitted before mlp(c)
                do_gather(0, (0, 1, 2, 3))
                do_gather(1, (0, 1, 2, 3))
                do_front(0)
                for c in range(NCHUNK):
                    if c + 2 < NCHUNK:
                        do_gather(c + 2, (0, 1))
                    do_mlp(c)
                    if c + 2 < NCHUNK:
                        do_gather(c + 2, (2, 3))
                    if c + 1 < NCHUNK:
                        do_front(c + 1)
                    do_corr(c)
                    if c >= 1:
                        do_scatter(c - 1)
                do_scatter(NCHUNK - 1)

    nc.compile()
    return nc


def _get_compiled():
    global _compiled
    if _compiled is None:
        _compiled = _build()
    return _compiled


# ------------------------------ entry point -------------------------------

def _prep_in_maps(coords, propers, encoded, t, answer, W0, b0, W1, b1, W2, b2,
                  W3, b3):
    coords = np.asarray(coords, dtype=np.float32)
    propers_np = np.asarray(propers)
    encoded = np.asarray(encoded, dtype=np.float32)
    t = np.asarray(t, dtype=np.float32)
    W0 = np.asarray(W0, dtype=np.float32)
    b0 = np.asarray(b0, dtype=np.float32)
    W1 = np.asarray(W1, dtype=np.float32)
    b1 = np.asarray(b1, dtype=np.float32)
    W2 = np.asarray(W2, dtype=np.float32)
    b2 = np.asarray(b2, dtype=np.float32)
    W3 = np.asarray(W3, dtype=np.float32)
    b3 = np.asarray(b3, dtype=np.float32)

    eslab = np.zeros((NA, 128), dtype=_BF16)
    eslab[:N_ATOMS] = encoded.astype(_BF16)
    cslab = np.zeros((NA, 64), dtype=np.float32)
    cslab[:N_ATOMS, 0:12] = coords.reshape(N_ATOMS, 12)

    w0all = np.stack([W0[128 * k:128 * (k + 1)] for k in range(4)],
                     axis=1).astype(_BF16)
    # wg4[4t+f, ti, :] selects time ti's geo features only
    wg4 = np.zeros((16, 4, 128), dtype=_BF16)
    for ti in range(T_STEPS):
        wg4[4 * ti + 0, ti] = W0[513].astype(_BF16)   # sin
        wg4[4 * ti + 1, ti] = W0[514].astype(_BF16)   # cos
        wg4[4 * ti + 2, ti] = W0[515].astype(_BF16)   # dl
    bx1 = np.zeros((D, 4), dtype=np.float32)
    for ti in range(T_STEPS):
        bx1[:, ti] = b0 + t[ti] * W0[512]
    bias12 = np.stack([b1, b2], axis=1).astype(np.float32)
    b3h = np.zeros((D, 2), dtype=np.float32)
    b3h[:, 0] = -0.5 * b3[0]
    b3h[:, 1] = 0.5 * b3[1]

    shared = {
        "eslab": eslab,
        "cslab": cslab,
        "w0all": w0all,
        "wg4": wg4,
        "w1": W1.astype(_BF16),
        "w2": W2.astype(_BF16),
        "w3": W3.astype(_BF16),
        "bx1": bx1,
        "bias12": bias12,
        "b3h": b3h,
    }

    props32 = propers_np.astype(np.int32)
    in_maps = []
    for cidx in range(N_CORES):
        shard = np.zeros((PPCT, 4), dtype=np.int32)
        shard[:PPC] = props32[cidx * PPC:(cidx + 1) * PPC]
        order = _order_props(shard, PPC, seed=cidx)
        po = shard[order]
        is_pad = order >= PPC
        gi = np.concatenate([_wrap_idxs(po[:, k]) for k in range(4)], axis=1)
        tgt0 = np.where(is_pad, DUMP, po[:, 0]).astype(np.int32)
        tgt3 = np.where(is_pad, DUMP, po[:, 3]).astype(np.int32)
        si = np.concatenate([_wrap_idxs(tgt0), _wrap_idxs(tgt3)], axis=1)
        in_maps.append({**shared, "gidx": gi, "sidx": si})
    return in_maps


def kernel(coords, propers, encoded, t, answer, W0, b0, W1, b1, W2, b2, W3, b3,
           _trace=False):
    from concourse.bass_utils import run_bass_kernel_spmd

    answer = np.asarray(answer, dtype=np.float32)
    in_maps = _prep_in_maps(coords, propers, encoded, t, answer, W0, b0, W1,
                            b1, W2, b2, W3, b3)
    nc = _get_compiled()
    res = run_bass_kernel_spmd(nc, in_maps, core_ids=list(range(N_CORES)),
                               trace=_trace)
    if _trace:
        kernel.last_exec_ns = res.exec_time_ns
        kernel.last_results = res

    acc = np.zeros((N_ATOMS, 12), dtype=np.float32)
    for cidx in range(N_CORES):
        for name in ("A0", "A3", "A0b", "A3b"):
            acc += res.results[cidx][name][:N_ATOMS, :12]
    out = answer + acc.reshape(N_ATOMS, T_STEPS, 3)
    return out.astype(np.float32)


kernel.last_exec_ns = None
kernel.last_results = None
